# revision 16
# baseline (speedup 1.0000x reference)
"""DeepseekMoE layer on 8 Trainium2 NeuronCores (Bass/Tile, expert-parallel).

Sharding (per the expert-parallel hint):
  - 16 routed experts -> 2 per core, paired big+small by token count so the
    per-core slot totals balance; token dispatch (all-to-all) is emulated at
    the sharding layer: the host computes the discrete top-4 routing, gathers
    each expert's tokens into a compact transposed batch, and scatter-adds the
    compact expert outputs back into the full output ("combine").
  - Shared expert is tensor-parallel over its intermediate dim (2816/8 = 352
    columns per core, padded to 384); the 8 partial outputs are summed on
    gather.
  - Gate (softmax + renormalized top-4 combine weights) is replicated and
    computed ON DEVICE from the hidden states; the host only supplies the
    discrete 0/1 top-4 mask (routing decision) and gather indices.

All FLOPs that produce output values run on device.  Matmul operands are
bf16 (fp32 PSUM accumulation): on TRN2 the PE runs bf16 and fp32r at the
same 1 row/cycle, so bf16 costs no compute but halves the HBM weight/
activation traffic, which is what bounds this kernel.  Only the tiny gate
softmax pipeline stays fp32.

Weights are host-packed into stationary-tile-major layout ([m-tile,
partition, k-tile, col]); gate+up (and consecutive down-proj m-tiles) are
further interleaved so one DMA instruction streams 0.7-1MB with 4KB
descriptors, keeping the DMA issue queues short.
"""

import os
import numpy as np

H = 2048          # hidden size
E = 16            # routed experts
TOPK = 4
I = 1408          # routed expert intermediate
ISH = 2816        # shared expert intermediate
T = 1024          # tokens
P = 128
NCORES = 8
EPC = 2           # experts per core
ISS = ISH // NCORES                  # 352 shared columns per core
ISSP = 384                           # padded to 3 full 128-tiles
KH = H // P                          # 16 k-tiles over H
MI = I // P                          # 11 m-tiles over I
MH = H // P                          # 16 m-tiles over H
KI = I // P                          # 11 k-tiles over I
KS = ISSP // P                       # 3 k-tiles over padded shared slice
ZERO_ROW_FLAT = T * E                # flat index of the zeroed scratch row

_NC_CACHE = {}
LAST_RESULTS = None  # BassKernelResults of the most recent run (for test.py)


def _bf16():
    import ml_dtypes
    return ml_dtypes.bfloat16


def _token_chunks(C):
    """Split [0, C) into matmul moving-dim chunks of <=512."""
    out = []
    off = 0
    while off < C:
        sz = min(512, C - off)
        out.append((off, sz))
        off += sz
    return out


def _pack_st(w, KT, MT):
    """[KT*P, MT*P] -> [MT*P, KT*P] tile-major stationary pack.

    packed[m*P + p, k*P + c] = w[k*P + p, m*P + c], so the device loads
    rows [m*P, (m+1)*P) as one [P, KT*P] block whose column-slice k is the
    stationary tile for (k, m).
    """
    return np.ascontiguousarray(
        w.reshape(KT, P, MT, P).transpose(2, 1, 0, 3).reshape(MT * P, KT * P))


def _concat_cols(packs, MT, KTP):
    """n [MT*P, KTP] packs -> [MT*P, n*KTP]: row (m*P+p) = concat of each pack's
    (m,p) row, so one contiguous-row DMA (hardware DGE, no rearrange) streams
    all n stationary blocks for m-tile m."""
    return np.ascontiguousarray(
        np.concatenate([w.reshape(MT, P, KTP) for w in packs], axis=2)
        .reshape(MT * P, len(packs) * KTP))


def _group_rows(w, MT, KTP, G):
    """[MT*P, KTP] -> [(MT//G)*P, G*KTP]: row (g*P+p) = concat of m-tiles
    g*G..g*G+G-1's row p, so G m-tiles load as one contiguous-row DMA."""
    return _concat_cols([w.reshape(MT, P, KTP)[i::G].reshape(MT // G * P, KTP)
                         for i in range(G)], MT // G, KTP)


def _build(CA, CB):
    import concourse.bacc as bacc
    import concourse.bass as bass
    import concourse.mybir as mybir
    import concourse.tile as tile
    from concourse.masks import make_identity

    f32 = mybir.dt.float32
    bf16 = mybir.dt.bfloat16
    i32 = mybir.dt.int32
    SILU = mybir.ActivationFunctionType.Silu
    EXP = mybir.ActivationFunctionType.Exp
    X = mybir.AxisListType.X

    CJ = [CA, CB]
    CHJ = [_token_chunks(CA), _token_chunks(CB)]
    COFFJ = [0, CA]        # column offsets in widx/zt
    CT = CA + CB
    NT = T // 512     # token chunks for shared/gate (2)

    nc = bacc.Bacc("TRN2", target_bir_lowering=False, debug=False)

    xt_h = nc.dram_tensor("xt", [(KH // 4) * P, 4 * T], bf16, kind="ExternalInput")
    gwtb_h = nc.dram_tensor("gwtb", [P, KH * E], bf16, kind="ExternalInput")
    maskb_h = nc.dram_tensor("maskb", [P, (T // P) * E], f32, kind="ExternalInput")
    xg_h = [nc.dram_tensor(f"xg{j}", [P, KH * CJ[j]], bf16, kind="ExternalInput") for j in range(EPC)]
    widx_h = nc.dram_tensor("widx", [CT, 1], i32, kind="ExternalInput")
    wgu_h = [nc.dram_tensor(f"wgu{j}", [MI * P, 2 * KH * P], bf16, kind="ExternalInput") for j in range(EPC)]
    wd_h = [nc.dram_tensor(f"wd{j}", [(MH // 2) * P, 2 * KI * P], bf16, kind="ExternalInput") for j in range(EPC)]
    swgu_h = nc.dram_tensor("swgu", [KS * P, 2 * KH * P], bf16, kind="ExternalInput")
    swd_h = nc.dram_tensor("swd", [(MH // 4) * P, 4 * KS * P], bf16, kind="ExternalInput")
    zt_h = nc.dram_tensor("zt", [H, CT], bf16, kind="ExternalOutput")
    st_h = nc.dram_tensor("st", [H, T], bf16, kind="ExternalOutput")

    with tile.TileContext(nc) as tc:
        with (
            tc.tile_pool(name="resident", bufs=1) as res_pool,
            tc.tile_pool(name="xgp", bufs=1) as xg_pool,
            tc.tile_pool(name="acts", bufs=1) as act_pool,
            tc.tile_pool(name="wstream", bufs=3) as wst_pool,
            tc.tile_pool(name="dstream", bufs=3) as dst_pool,
            tc.tile_pool(name="sstream", bufs=2) as sst_pool,
            tc.tile_pool(name="small", bufs=2) as small_pool,
            tc.tile_pool(name="stage", bufs=3) as stage_pool,
            tc.tile_pool(name="ps", bufs=1, space="PSUM") as ps_pool,
            tc.tile_pool(name="dram", bufs=1, space="DRAM") as dram_pool,
        ):
            # ---------------- resident loads ----------------
            # xgb0 (gpsimd queue) + first wgu tiles (sync queue) land first so
            # upgate(0) starts the PE ~5us in; xt and xgb1 are issued from
            # inside the upgate(0) m-loop so they don't steal HBM bandwidth
            # from the weight stream during startup (gate needs xt only ~50us
            # in, upgate(1) needs xgb1 ~130us in).
            xgb = [xg_pool.tile([P, KH * CJ[j]], bf16, name=f"xgb{j}", tag="xgb") for j in range(EPC)]
            nc.gpsimd.dma_start(xgb[0][:], xg_h[0][:])
            xt4 = [res_pool.tile([P, 4 * T], bf16, name=f"xt4_{kk}", tag=f"xt4_{kk}") for kk in range(KH // 4)]

            def load_xt(kk):
                nc.gpsimd.dma_start(xt4[kk][:], xt_h[kk * P:(kk + 1) * P, :])

            xt_t = [xt4[k // 4][:, (k % 4) * T:(k % 4 + 1) * T] for k in range(KH)]
            gwtb = res_pool.tile([P, KH * E], bf16, name="gwtb", tag="gwtb")
            nc.gpsimd.dma_start(gwtb[:], gwtb_h[:])
            maskb = res_pool.tile([P, (T // P) * E], f32, name="maskb", tag="maskb")
            nc.gpsimd.dma_start(maskb[:], maskb_h[:])
            ident = res_pool.tile([P, P], f32, name="ident", tag="ident")
            make_identity(nc, ident[:])
            zbias = res_pool.tile([P, 1], f32, name="zbias", tag="zbias")
            nc.vector.memset(zbias[:], 0.0)

            # combine-weight scratch in HBM: rows 0..T-1 = combine, row T = zeros
            wflat = dram_pool.tile([(T + 1) * E, 1], f32, name="wflat")
            wflat2d = wflat[:].rearrange("(a b) o -> a (b o)", b=E)
            zrow = res_pool.tile([1, E], f32, name="zrow", tag="zrow")
            nc.vector.memset(zrow[:], 0.0)
            nc.gpsimd.dma_start(wflat2d[T:T + 1, :], zrow[:])

            wb = [res_pool.tile([P, CJ[j]], f32, name=f"wb{j}", tag=f"wb{j}") for j in range(EPC)]
            a_t = [[act_pool.tile([P, CJ[j]], bf16, name=f"a{j}_{m}", tag=f"a{j}_{m}") for m in range(MI)]
                   for j in range(EPC)]
            sg_t = [act_pool.tile([P, T], f32, name=f"sg{m}", tag="sgtmp", bufs=2) for m in range(KS)]
            as_t = [act_pool.tile([P, T], bf16, name=f"as{m}", tag=f"as{m}") for m in range(KS)]

            # ---------------- emission sections ----------------
            def emit_gate():
                lgps = ps_pool.tile([E, T], f32, name="lgps", tag="B1", bufs=2)
                for n in range(NT):
                    for k in range(KH):
                        nc.tensor.matmul(
                            lgps[:, n * 512:(n + 1) * 512],
                            lhsT=gwtb[:, k * E:(k + 1) * E],
                            rhs=xt_t[k][:, n * 512:(n + 1) * 512],
                            start=(k == 0), stop=(k == KH - 1),
                        )
                lgsb = res_pool.tile([E, T], f32, name="lgsb", tag="lgsb")
                nc.scalar.copy(lgsb[:], lgps[:])
                for t8 in range(T // P):
                    trps = ps_pool.tile([P, E], f32, name=f"tr{t8}", tag="A1", bufs=4)
                    nc.tensor.transpose(
                        out=trps[:], in_=lgsb[:, t8 * P:(t8 + 1) * P], identity=ident[0:E, 0:E],
                    )
                    sc = small_pool.tile([P, E], f32, name=f"sc{t8}", tag="sc")
                    nc.scalar.activation(sc[:], trps[:], EXP, bias=zbias[:])
                    mskd = small_pool.tile([P, E], f32, name=f"mskd{t8}", tag="mskd")
                    nc.vector.tensor_mul(out=mskd[:], in0=sc[:], in1=maskb[:, t8 * E:(t8 + 1) * E])
                    ssum = small_pool.tile([P, 1], f32, name=f"ssum{t8}", tag="ssum")
                    nc.vector.reduce_sum(ssum[:], mskd[:], axis=X)
                    rsum = small_pool.tile([P, 1], f32, name=f"rsum{t8}", tag="rsum")
                    nc.vector.reciprocal(rsum[:], ssum[:])
                    comb = small_pool.tile([P, E], f32, name=f"comb{t8}", tag="comb")
                    nc.vector.tensor_scalar_mul(comb[:], mskd[:], rsum[:, :1])
                    nc.gpsimd.dma_start(wflat2d[t8 * P:(t8 + 1) * P, :], comb[:])

            def emit_gather(j):
                # per-slot combine weights -> partition-broadcast wb[j]
                off = 0
                while off < CJ[j]:
                    csz = min(P, CJ[j] - off)
                    it = small_pool.tile([P, 1], i32, name=f"it{j}_{off}", tag="it")
                    nc.gpsimd.dma_start(it[:csz], widx_h[COFFJ[j] + off:COFFJ[j] + off + csz, :])
                    wslot = small_pool.tile([P, 1], f32, name=f"ws{j}_{off}", tag="ws")
                    nc.gpsimd.indirect_dma_start(
                        out=wslot[:csz, :], out_offset=None, in_=wflat[:],
                        in_offset=bass.IndirectOffsetOnAxis(ap=it[:csz, :1], axis=0),
                    )
                    wbps = ps_pool.tile([P, P], f32, name=f"wbps{j}_{off}", tag="A1", bufs=4)
                    nc.tensor.transpose(
                        out=wbps[:, :csz],
                        in_=wslot[:csz, :1].to_broadcast([csz, P]),
                        identity=ident[0:csz, 0:csz],
                    )
                    nc.vector.tensor_copy(wb[j][:, off:off + csz], wbps[:, :csz])
                    off += csz

            def emit_upgate(j, interleave=None):
                xg_t = [xgb[j][:, k * CJ[j]:(k + 1) * CJ[j]] for k in range(KH)]
                g_t = [act_pool.tile([P, CJ[j]], f32, name=f"g{j}_{m}", tag="gtmp", bufs=3) for m in range(MI)]
                for m in range(MI):
                    if interleave is not None:
                        interleave(m)
                    wgub = wst_pool.tile([P, 2 * KH * P], bf16, name=f"wgub{j}_{m}", tag="wblk", bufs=3)
                    nc.sync.dma_start(wgub[:], wgu_h[j][m * P:(m + 1) * P, :])
                    wgb = [wgub[:, k * P:(k + 1) * P] for k in range(KH)]
                    wub = [wgub[:, (KH + k) * P:(KH + k + 1) * P] for k in range(KH)]
                    for (coff, csz) in CHJ[j]:
                        psg = ps_pool.tile([P, csz], f32, name=f"psg{j}_{m}_{coff}", tag="A1", bufs=4)
                        for k in range(KH):
                            nc.tensor.matmul(psg[:], lhsT=wgb[k],
                                             rhs=xg_t[k][:, coff:coff + csz],
                                             start=(k == 0), stop=(k == KH - 1))
                        nc.scalar.activation(g_t[m][:, coff:coff + csz], psg[:], SILU, bias=zbias[:])
                        psu = ps_pool.tile([P, csz], f32, name=f"psu{j}_{m}_{coff}", tag="A1", bufs=4)
                        for k in range(KH):
                            nc.tensor.matmul(psu[:], lhsT=wub[k],
                                             rhs=xg_t[k][:, coff:coff + csz],
                                             start=(k == 0), stop=(k == KH - 1))
                        # a = silu(g) * u straight out of PSUM, rounded to bf16
                        nc.vector.tensor_mul(out=a_t[j][m][:, coff:coff + csz],
                                             in0=g_t[m][:, coff:coff + csz], in1=psu[:])

            def emit_down(j, interleave=None):
                for mg in range(MH // 2):
                    if interleave is not None:
                        interleave(mg)
                    wdb = dst_pool.tile([P, 2 * KI * P], bf16, name=f"wdb{j}_{mg}", tag="wdb", bufs=3)
                    nc.sync.dma_start(wdb[:], wd_h[j][mg * P:(mg + 1) * P, :])
                    for mh in range(2):
                        m = mg * 2 + mh
                        for (coff, csz) in CHJ[j]:
                            psz = ps_pool.tile([P, csz], f32, name=f"psz{j}_{m}_{coff}", tag="A1", bufs=4)
                            for k in range(KI):
                                nc.tensor.matmul(psz[:], lhsT=wdb[:, (mh * KI + k) * P:(mh * KI + k + 1) * P],
                                                 rhs=a_t[j][k][:, coff:coff + csz],
                                                 start=(k == 0), stop=(k == KI - 1))
                            zst = stage_pool.tile([P, csz], bf16, name=f"zst{j}_{m}_{coff}", tag="zst", bufs=2)
                            # combine-weight scaling fused into the eviction
                            nc.vector.tensor_mul(out=zst[:], in0=wb[j][:, coff:coff + csz], in1=psz[:])
                            nc.gpsimd.dma_start(
                                zt_h[m * P:(m + 1) * P, COFFJ[j] + coff:COFFJ[j] + coff + csz], zst[:])

            def emit_shared_ug(mi):
                sgub = sst_pool.tile([P, 2 * KH * P], bf16, name=f"sgub{mi}", tag="ssb", bufs=2)
                nc.sync.dma_start(sgub[:], swgu_h[mi * P:(mi + 1) * P, :])
                psgs = ps_pool.tile([P, T], f32, name=f"psgs{mi}", tag="B1", bufs=2)
                for k in range(KH):
                    for n in range(NT):
                        nc.tensor.matmul(psgs[:, n * 512:(n + 1) * 512],
                                         lhsT=sgub[:, k * P:(k + 1) * P],
                                         rhs=xt_t[k][:, n * 512:(n + 1) * 512],
                                         start=(k == 0), stop=(k == KH - 1))
                nc.scalar.activation(sg_t[mi][:], psgs[:], SILU, bias=zbias[:])
                psus = ps_pool.tile([P, T], f32, name=f"psus{mi}", tag="B1", bufs=2)
                for k in range(KH):
                    for n in range(NT):
                        nc.tensor.matmul(psus[:, n * 512:(n + 1) * 512],
                                         lhsT=sgub[:, (KH + k) * P:(KH + k + 1) * P],
                                         rhs=xt_t[k][:, n * 512:(n + 1) * 512],
                                         start=(k == 0), stop=(k == KH - 1))
                nc.vector.tensor_mul(out=as_t[mi][:], in0=sg_t[mi][:], in1=psus[:])

            def emit_shared_down(ms):
                for mg in ms:
                    sdb = sst_pool.tile([P, 4 * KS * P], bf16, name=f"sdb{mg}", tag="sdb", bufs=2)
                    nc.sync.dma_start(sdb[:], swd_h[mg * P:(mg + 1) * P, :])
                    for mh in range(4):
                        m = mg * 4 + mh
                        psys = ps_pool.tile([P, T], f32, name=f"psys{m}", tag="B1", bufs=2)
                        for ki in range(KS):
                            for n in range(NT):
                                nc.tensor.matmul(psys[:, n * 512:(n + 1) * 512],
                                                 lhsT=sdb[:, (mh * KS + ki) * P:(mh * KS + ki + 1) * P],
                                                 rhs=as_t[ki][:, n * 512:(n + 1) * 512],
                                                 start=(ki == 0), stop=(ki == KS - 1))
                        sstg = stage_pool.tile([P, T], bf16, name=f"sstg{m}", tag="sstage", bufs=2)
                        # alternate the PSUM eviction engine so neither
                        # scalar nor vector saturates during the tail
                        if m % 2 == 0:
                            nc.scalar.copy(sstg[:], psys[:])
                        else:
                            nc.vector.tensor_copy(sstg[:], psys[:])
                        nc.gpsimd.dma_start(st_h[m * P:(m + 1) * P, :], sstg[:])

            # PE-section order: start with upgate(0) (only needs xgb0 + the
            # first wgu tile, ~2.3MB); gate runs after it, by which time xt
            # has streamed in on the scalar queue.  PE-light shared sections
            # are interleaved into the DMA-heavy routed sections so the
            # weight-stream demand stays below the ~330GB/s HBM rate.
            def upg0_ilv(m):
                # stagger the xt / xgb1 resident loads through the m-loop
                if m in (1, 3, 5, 7):
                    load_xt(m // 2)
                elif m == 9:
                    nc.gpsimd.dma_start(xgb[1][:], xg_h[1][:])

            emit_upgate(0, interleave=upg0_ilv)
            emit_gate()
            emit_shared_ug(0)
            emit_gather(0)
            emit_gather(1)
            emit_down(0, interleave=lambda mg: emit_shared_ug(1) if mg == 2 else None)
            emit_upgate(1, interleave=lambda m: emit_shared_ug(2) if m == 5 else None)
            emit_down(1, interleave=lambda mg: emit_shared_down([mg]) if mg < 4 else None)

    nc.compile()
    return nc


def _get_nc(CA, CB):
    if (CA, CB) not in _NC_CACHE:
        _NC_CACHE[(CA, CB)] = _build(CA, CB)
    return _NC_CACHE[(CA, CB)]


def kernel(**inputs):
    global LAST_RESULTS
    from concourse.bass_utils import run_bass_kernel_spmd

    BF16 = _bf16()

    hs = np.asarray(inputs["hidden_states"], dtype=np.float32)
    gate_w = np.asarray(inputs["gate_w"], dtype=np.float32)
    w_gate = np.asarray(inputs["w_gate"], dtype=np.float32)
    w_up = np.asarray(inputs["w_up"], dtype=np.float32)
    w_down = np.asarray(inputs["w_down"], dtype=np.float32)
    sw_gate = np.asarray(inputs["sw_gate"], dtype=np.float32)
    sw_up = np.asarray(inputs["sw_up"], dtype=np.float32)
    sw_down = np.asarray(inputs["sw_down"], dtype=np.float32)

    orig_shape = hs.shape
    x = hs.reshape(-1, H)
    assert x.shape[0] == T

    # ---- host: discrete routing only (top-4 selection + dispatch tables) ----
    logits = x @ gate_w.T
    smax = logits.max(axis=-1, keepdims=True)
    sc = np.exp(logits - smax)
    sc /= sc.sum(axis=-1, keepdims=True)
    order = np.argsort(-sc, axis=-1, kind="stable")[:, :TOPK]
    mask = np.zeros((T, E), dtype=np.float32)
    mask[np.arange(T)[:, None], order] = 1.0
    tok_lists = [np.nonzero(mask[:, e])[0].astype(np.int64) for e in range(E)]

    # balance: pair the i-th most-loaded expert with the i-th least-loaded
    sizes = np.array([len(tk) for tk in tok_lists])
    by_load = np.argsort(-sizes, kind="stable")
    pairs = [(int(by_load[i]), int(by_load[E - 1 - i])) for i in range(NCORES)]
    CA = max(64, int(np.ceil(max(sizes[p[0]] for p in pairs) / 32)) * 32)
    CB = max(64, int(np.ceil(max(sizes[p[1]] for p in pairs) / 32)) * 32)
    CJ = [CA, CB]

    nc = _get_nc(CA, CB)

    xT = np.ascontiguousarray(x.T)
    xTb = xT.astype(BF16)
    # xt packed for 4-ktile row loads: xtp[kk*P + p, a*T + t] = x[t, (4kk+a)*P + p]
    xtp = _group_rows(xTb, KH, T, 4)
    # gate weights packed: gwtb[p, k*E + e] = gate_w[e, k*P + p]
    gwtb = np.ascontiguousarray(
        gate_w.T.reshape(KH, P, E).transpose(1, 0, 2).reshape(P, KH * E)).astype(BF16)
    # mask packed: maskb[p, t8*E + e] = mask[t8*P + p, e]
    maskb = np.ascontiguousarray(mask.reshape(T // P, P, E).transpose(1, 0, 2).reshape(P, (T // P) * E))

    # shared slices, zero-padded to 384 and tile-major packed
    def pad_cols(w, newc):
        out = np.zeros((w.shape[0], newc), dtype=np.float32)
        out[:, :w.shape[1]] = w
        return out

    def pad_rows(w, newr):
        out = np.zeros((newr, w.shape[1]), dtype=np.float32)
        out[:w.shape[0], :] = w
        return out

    in_maps = []
    for c in range(NCORES):
        es = pairs[c]
        widx = np.full((CA + CB, 1), ZERO_ROW_FLAT, dtype=np.int32)
        sg_p = _pack_st(pad_cols(sw_gate[:, c * ISS:(c + 1) * ISS], ISSP), KH, KS)
        su_p = _pack_st(pad_cols(sw_up[:, c * ISS:(c + 1) * ISS], ISSP), KH, KS)
        im = {
            "xt": xtp, "gwtb": gwtb, "maskb": maskb, "widx": widx,
            "swgu": _concat_cols([sg_p, su_p], KS, KH * P).astype(BF16),
            "swd": _group_rows(
                _pack_st(pad_rows(sw_down[c * ISS:(c + 1) * ISS, :], ISSP), KS, MH),
                MH, KS * P, 4).astype(BF16),
        }
        coff = 0
        for j, e in enumerate(es):
            tk = tok_lists[e]
            C = CJ[j]
            widx[coff:coff + len(tk), 0] = (tk * E + e).astype(np.int32)
            coff += C
            # gathered activations, tile-major: xg[p, k*C + c] = x[tok_c, k*P + p]
            xg = np.zeros((P, KH * C), dtype=BF16)
            g = xTb[:, tk].reshape(KH, P, len(tk)).transpose(1, 0, 2)  # [P, KH, n]
            xg.reshape(P, KH, C)[:, :, :len(tk)] = g
            im[f"xg{j}"] = xg
            wg_p = _pack_st(w_gate[e], KH, MI)
            wu_p = _pack_st(w_up[e], KH, MI)
            im[f"wgu{j}"] = _concat_cols([wg_p, wu_p], MI, KH * P).astype(BF16)
            im[f"wd{j}"] = _group_rows(_pack_st(w_down[e], KI, MH), MH, KI * P, 2).astype(BF16)
        in_maps.append(im)

    trace = bool(int(os.environ.get("BASSMOE_TRACE", "0")))
    kwargs = {}
    if trace:
        kwargs = dict(trace=True, tmpdir=os.environ.get("BASSMOE_TRACE_DIR") or None)
        tcores = os.environ.get("BASSMOE_TRACE_CORES")
        if tcores:
            kwargs["trace_cores"] = [int(x) for x in tcores.split(",")]
            kwargs["stitch_traces"] = False
    res = run_bass_kernel_spmd(nc, in_maps, core_ids=list(range(NCORES)), **kwargs)
    LAST_RESULTS = res

    # ---- host: unshard (scatter-add compact expert outputs + sum partials) ----
    y = np.zeros((T, H), dtype=np.float64)
    st_sum = np.zeros((H, T), dtype=np.float64)
    for c in range(NCORES):
        r = res.results[c]
        st_sum += np.asarray(r["st"], dtype=np.float64)
        coff = 0
        for j, e in enumerate(pairs[c]):
            tk = tok_lists[e]
            zt = np.asarray(r["zt"], dtype=np.float64)
            y[tk] += zt[:, coff:coff + len(tk)].T
            coff += CJ[j]
    y += st_sum.T
    return y.astype(np.float32).reshape(orig_shape)


# revision 20
# speedup vs baseline: 1.0223x; 1.0223x over previous
"""DeepseekMoE layer on 8 Trainium2 NeuronCores (Bass/Tile, expert-parallel).

Sharding (per the expert-parallel hint):
  - 16 routed experts -> 2 per core, paired big+small by token count so the
    per-core slot totals balance; token dispatch (all-to-all) is emulated at
    the sharding layer: the host computes the discrete top-4 routing, gathers
    each expert's tokens into a compact transposed batch, and scatter-adds the
    compact expert outputs back into the full output ("combine").
  - Shared expert is tensor-parallel over its intermediate dim (2816/8 = 352
    columns per core, padded to 384); the 8 partial outputs are summed on
    gather.
  - Gate (softmax + renormalized top-4 combine weights) is replicated and
    computed ON DEVICE from the hidden states; the host only supplies the
    discrete 0/1 top-4 mask (routing decision) and gather indices.

All FLOPs that produce output values run on device.  Matmul operands are
bf16 (fp32 PSUM accumulation): on TRN2 the PE runs bf16 and fp32r at the
same 1 row/cycle, so bf16 costs no compute but halves the HBM weight/
activation traffic, which is what bounds this kernel.  Only the tiny gate
softmax pipeline stays fp32.

Weights are host-packed into stationary-tile-major layout ([m-tile,
partition, k-tile, col]); gate+up (and consecutive down-proj m-tiles) are
further interleaved so one DMA instruction streams 0.7-1MB with 4KB
descriptors, keeping the DMA issue queues short.
"""

import os
import numpy as np

H = 2048          # hidden size
E = 16            # routed experts
TOPK = 4
I = 1408          # routed expert intermediate
ISH = 2816        # shared expert intermediate
T = 1024          # tokens
P = 128
NCORES = 8
EPC = 2           # experts per core
ISS = ISH // NCORES                  # 352 shared columns per core
ISSP = 384                           # padded to 3 full 128-tiles
KH = H // P                          # 16 k-tiles over H
MI = I // P                          # 11 m-tiles over I
MH = H // P                          # 16 m-tiles over H
KI = I // P                          # 11 k-tiles over I
KS = ISSP // P                       # 3 k-tiles over padded shared slice
ZERO_ROW_FLAT = T * E                # flat index of the zeroed scratch row

_NC_CACHE = {}
LAST_RESULTS = None  # BassKernelResults of the most recent run (for test.py)


def _bf16():
    import ml_dtypes
    return ml_dtypes.bfloat16


def _token_chunks(C):
    """Split [0, C) into matmul moving-dim chunks of <=512."""
    out = []
    off = 0
    while off < C:
        sz = min(512, C - off)
        out.append((off, sz))
        off += sz
    return out


def _pack_st(w, KT, MT):
    """[KT*P, MT*P] -> [MT*P, KT*P] tile-major stationary pack.

    packed[m*P + p, k*P + c] = w[k*P + p, m*P + c], so the device loads
    rows [m*P, (m+1)*P) as one [P, KT*P] block whose column-slice k is the
    stationary tile for (k, m).
    """
    return np.ascontiguousarray(
        w.reshape(KT, P, MT, P).transpose(2, 1, 0, 3).reshape(MT * P, KT * P))


def _concat_cols(packs, MT, KTP):
    """n [MT*P, KTP] packs -> [MT*P, n*KTP]: row (m*P+p) = concat of each pack's
    (m,p) row, so one contiguous-row DMA (hardware DGE, no rearrange) streams
    all n stationary blocks for m-tile m."""
    return np.ascontiguousarray(
        np.concatenate([w.reshape(MT, P, KTP) for w in packs], axis=2)
        .reshape(MT * P, len(packs) * KTP))


def _group_rows(w, MT, KTP, G):
    """[MT*P, KTP] -> [(MT//G)*P, G*KTP]: row (g*P+p) = concat of m-tiles
    g*G..g*G+G-1's row p, so G m-tiles load as one contiguous-row DMA."""
    return _concat_cols([w.reshape(MT, P, KTP)[i::G].reshape(MT // G * P, KTP)
                         for i in range(G)], MT // G, KTP)


def _build(CA, CB):
    import concourse.bacc as bacc
    import concourse.bass as bass
    import concourse.mybir as mybir
    import concourse.tile as tile
    from concourse.masks import make_identity

    f32 = mybir.dt.float32
    bf16 = mybir.dt.bfloat16
    i32 = mybir.dt.int32
    SILU = mybir.ActivationFunctionType.Silu
    EXP = mybir.ActivationFunctionType.Exp
    X = mybir.AxisListType.X

    CJ = [CA, CB]
    CHJ = [_token_chunks(CA), _token_chunks(CB)]
    COFFJ = [0, CA]        # column offsets in widx/zt
    CT = CA + CB
    NT = T // 512     # token chunks for shared/gate (2)

    nc = bacc.Bacc("TRN2", target_bir_lowering=False, debug=False)

    xt_h = nc.dram_tensor("xt", [(KH // 4) * P, 4 * T], bf16, kind="ExternalInput")
    gwtb_h = nc.dram_tensor("gwtb", [P, KH * E], bf16, kind="ExternalInput")
    maskb_h = nc.dram_tensor("maskb", [P, (T // P) * E], f32, kind="ExternalInput")
    xg_h = [nc.dram_tensor(f"xg{j}", [P, KH * CJ[j]], bf16, kind="ExternalInput") for j in range(EPC)]
    widx_h = nc.dram_tensor("widx", [CT, 1], i32, kind="ExternalInput")
    wgu_h = [nc.dram_tensor(f"wgu{j}", [MI * P, 2 * KH * P], bf16, kind="ExternalInput") for j in range(EPC)]
    wd_h = [nc.dram_tensor(f"wd{j}", [(MH // 2) * P, 2 * KI * P], bf16, kind="ExternalInput") for j in range(EPC)]
    swgu_h = nc.dram_tensor("swgu", [KS * P, 2 * KH * P], bf16, kind="ExternalInput")
    swd_h = nc.dram_tensor("swd", [(MH // 4) * P, 4 * KS * P], bf16, kind="ExternalInput")
    zt_h = nc.dram_tensor("zt", [H, CT], bf16, kind="ExternalOutput")
    st_h = nc.dram_tensor("st", [H, T], bf16, kind="ExternalOutput")

    with tile.TileContext(nc) as tc:
        with (
            tc.tile_pool(name="resident", bufs=1) as res_pool,
            tc.tile_pool(name="xgp", bufs=1) as xg_pool,
            tc.tile_pool(name="acts", bufs=1) as act_pool,
            tc.tile_pool(name="wstream", bufs=3) as wst_pool,
            tc.tile_pool(name="dstream", bufs=3) as dst_pool,
            tc.tile_pool(name="sstream", bufs=2) as sst_pool,
            tc.tile_pool(name="small", bufs=2) as small_pool,
            tc.tile_pool(name="stage", bufs=3) as stage_pool,
            tc.tile_pool(name="ps", bufs=1, space="PSUM") as ps_pool,
            tc.tile_pool(name="dram", bufs=1, space="DRAM") as dram_pool,
        ):
            # ---------------- resident loads ----------------
            # xgb0 (gpsimd queue) + first wgu tiles (sync queue) land first so
            # upgate(0) starts the PE ~5us in; xt and xgb1 are issued from
            # inside the upgate(0) m-loop so they don't steal HBM bandwidth
            # from the weight stream during startup (gate needs xt only ~50us
            # in, upgate(1) needs xgb1 ~130us in).
            xgb = [xg_pool.tile([P, KH * CJ[j]], bf16, name=f"xgb{j}", tag="xgb") for j in range(EPC)]
            nc.gpsimd.dma_start(xgb[0][:], xg_h[0][:])
            xt4 = [res_pool.tile([P, 4 * T], bf16, name=f"xt4_{kk}", tag=f"xt4_{kk}") for kk in range(KH // 4)]

            def load_xt(kk):
                # on the sync queue: DMA descriptors drain FIFO across all
                # issuing engines, so xt must enqueue BEHIND the wgu weight
                # tiles (whose issues are throttled by pool-buffer reuse) or
                # it steals the weight stream's HBM bandwidth at startup
                nc.sync.dma_start(xt4[kk][:], xt_h[kk * P:(kk + 1) * P, :])

            xt_t = [xt4[k // 4][:, (k % 4) * T:(k % 4 + 1) * T] for k in range(KH)]
            gwtb = res_pool.tile([P, KH * E], bf16, name="gwtb", tag="gwtb")
            nc.gpsimd.dma_start(gwtb[:], gwtb_h[:])
            maskb = res_pool.tile([P, (T // P) * E], f32, name="maskb", tag="maskb")
            nc.gpsimd.dma_start(maskb[:], maskb_h[:])
            ident = res_pool.tile([P, P], f32, name="ident", tag="ident")
            make_identity(nc, ident[:])
            zbias = res_pool.tile([P, 1], f32, name="zbias", tag="zbias")
            nc.vector.memset(zbias[:], 0.0)

            # combine-weight scratch in HBM: rows 0..T-1 = combine, row T = zeros
            wflat = dram_pool.tile([(T + 1) * E, 1], f32, name="wflat")
            wflat2d = wflat[:].rearrange("(a b) o -> a (b o)", b=E)
            zrow = res_pool.tile([1, E], f32, name="zrow", tag="zrow")
            nc.vector.memset(zrow[:], 0.0)
            nc.gpsimd.dma_start(wflat2d[T:T + 1, :], zrow[:])

            wb = [res_pool.tile([P, CJ[j]], f32, name=f"wb{j}", tag=f"wb{j}") for j in range(EPC)]
            a_t = [[act_pool.tile([P, CJ[j]], bf16, name=f"a{j}_{m}", tag=f"a{j}_{m}") for m in range(MI)]
                   for j in range(EPC)]
            sg_t = [act_pool.tile([P, T], f32, name=f"sg{m}", tag="sgtmp", bufs=2) for m in range(KS)]
            as_t = [act_pool.tile([P, T], bf16, name=f"as{m}", tag=f"as{m}") for m in range(KS)]

            # ---------------- emission sections ----------------
            def emit_gate():
                lgps = ps_pool.tile([E, T], f32, name="lgps", tag="B1", bufs=2)
                for n in range(NT):
                    for k in range(KH):
                        nc.tensor.matmul(
                            lgps[:, n * 512:(n + 1) * 512],
                            lhsT=gwtb[:, k * E:(k + 1) * E],
                            rhs=xt_t[k][:, n * 512:(n + 1) * 512],
                            start=(k == 0), stop=(k == KH - 1),
                        )
                lgsb = res_pool.tile([E, T], f32, name="lgsb", tag="lgsb")
                nc.scalar.copy(lgsb[:], lgps[:])
                for t8 in range(T // P):
                    trps = ps_pool.tile([P, E], f32, name=f"tr{t8}", tag="A1", bufs=4)
                    nc.tensor.transpose(
                        out=trps[:], in_=lgsb[:, t8 * P:(t8 + 1) * P], identity=ident[0:E, 0:E],
                    )
                    sc = small_pool.tile([P, E], f32, name=f"sc{t8}", tag="sc")
                    nc.scalar.activation(sc[:], trps[:], EXP, bias=zbias[:])
                    mskd = small_pool.tile([P, E], f32, name=f"mskd{t8}", tag="mskd")
                    nc.vector.tensor_mul(out=mskd[:], in0=sc[:], in1=maskb[:, t8 * E:(t8 + 1) * E])
                    ssum = small_pool.tile([P, 1], f32, name=f"ssum{t8}", tag="ssum")
                    nc.vector.reduce_sum(ssum[:], mskd[:], axis=X)
                    rsum = small_pool.tile([P, 1], f32, name=f"rsum{t8}", tag="rsum")
                    nc.vector.reciprocal(rsum[:], ssum[:])
                    comb = small_pool.tile([P, E], f32, name=f"comb{t8}", tag="comb")
                    nc.vector.tensor_scalar_mul(comb[:], mskd[:], rsum[:, :1])
                    nc.gpsimd.dma_start(wflat2d[t8 * P:(t8 + 1) * P, :], comb[:])

            def emit_gather(j):
                # per-slot combine weights -> partition-broadcast wb[j]
                off = 0
                while off < CJ[j]:
                    csz = min(P, CJ[j] - off)
                    it = small_pool.tile([P, 1], i32, name=f"it{j}_{off}", tag="it")
                    nc.gpsimd.dma_start(it[:csz], widx_h[COFFJ[j] + off:COFFJ[j] + off + csz, :])
                    wslot = small_pool.tile([P, 1], f32, name=f"ws{j}_{off}", tag="ws")
                    nc.gpsimd.indirect_dma_start(
                        out=wslot[:csz, :], out_offset=None, in_=wflat[:],
                        in_offset=bass.IndirectOffsetOnAxis(ap=it[:csz, :1], axis=0),
                    )
                    wbps = ps_pool.tile([P, P], f32, name=f"wbps{j}_{off}", tag="A1", bufs=4)
                    nc.tensor.transpose(
                        out=wbps[:, :csz],
                        in_=wslot[:csz, :1].to_broadcast([csz, P]),
                        identity=ident[0:csz, 0:csz],
                    )
                    nc.vector.tensor_copy(wb[j][:, off:off + csz], wbps[:, :csz])
                    off += csz

            def emit_upgate(j, interleave=None):
                xg_t = [xgb[j][:, k * CJ[j]:(k + 1) * CJ[j]] for k in range(KH)]
                g_t = [act_pool.tile([P, CJ[j]], f32, name=f"g{j}_{m}", tag="gtmp", bufs=3) for m in range(MI)]
                for m in range(MI):
                    if interleave is not None:
                        interleave(m)
                    wgub = wst_pool.tile([P, 2 * KH * P], bf16, name=f"wgub{j}_{m}", tag="wblk", bufs=4)
                    nc.sync.dma_start(wgub[:], wgu_h[j][m * P:(m + 1) * P, :])
                    wgb = [wgub[:, k * P:(k + 1) * P] for k in range(KH)]
                    wub = [wgub[:, (KH + k) * P:(KH + k + 1) * P] for k in range(KH)]
                    for (coff, csz) in CHJ[j]:
                        psg = ps_pool.tile([P, csz], f32, name=f"psg{j}_{m}_{coff}", tag="A1", bufs=4)
                        for k in range(KH):
                            nc.tensor.matmul(psg[:], lhsT=wgb[k],
                                             rhs=xg_t[k][:, coff:coff + csz],
                                             start=(k == 0), stop=(k == KH - 1))
                        nc.scalar.activation(g_t[m][:, coff:coff + csz], psg[:], SILU, bias=zbias[:])
                        psu = ps_pool.tile([P, csz], f32, name=f"psu{j}_{m}_{coff}", tag="A1", bufs=4)
                        for k in range(KH):
                            nc.tensor.matmul(psu[:], lhsT=wub[k],
                                             rhs=xg_t[k][:, coff:coff + csz],
                                             start=(k == 0), stop=(k == KH - 1))
                        # a = silu(g) * u straight out of PSUM, rounded to bf16
                        nc.vector.tensor_mul(out=a_t[j][m][:, coff:coff + csz],
                                             in0=g_t[m][:, coff:coff + csz], in1=psu[:])

            def emit_down(j, interleave=None):
                for mg in range(MH // 2):
                    if interleave is not None:
                        interleave(mg)
                    wdb = dst_pool.tile([P, 2 * KI * P], bf16, name=f"wdb{j}_{mg}", tag="wdb", bufs=3)
                    nc.sync.dma_start(wdb[:], wd_h[j][mg * P:(mg + 1) * P, :])
                    for mh in range(2):
                        m = mg * 2 + mh
                        for (coff, csz) in CHJ[j]:
                            psz = ps_pool.tile([P, csz], f32, name=f"psz{j}_{m}_{coff}", tag="A1", bufs=4)
                            for k in range(KI):
                                nc.tensor.matmul(psz[:], lhsT=wdb[:, (mh * KI + k) * P:(mh * KI + k + 1) * P],
                                                 rhs=a_t[j][k][:, coff:coff + csz],
                                                 start=(k == 0), stop=(k == KI - 1))
                            zst = stage_pool.tile([P, csz], bf16, name=f"zst{j}_{m}_{coff}", tag="zst", bufs=2)
                            # combine-weight scaling fused into the eviction
                            nc.vector.tensor_mul(out=zst[:], in0=wb[j][:, coff:coff + csz], in1=psz[:])
                            nc.gpsimd.dma_start(
                                zt_h[m * P:(m + 1) * P, COFFJ[j] + coff:COFFJ[j] + coff + csz], zst[:])

            def emit_shared_ug(mi):
                sgub = sst_pool.tile([P, 2 * KH * P], bf16, name=f"sgub{mi}", tag="ssb", bufs=2)
                nc.sync.dma_start(sgub[:], swgu_h[mi * P:(mi + 1) * P, :])
                psgs = ps_pool.tile([P, T], f32, name=f"psgs{mi}", tag="B1", bufs=2)
                for k in range(KH):
                    for n in range(NT):
                        nc.tensor.matmul(psgs[:, n * 512:(n + 1) * 512],
                                         lhsT=sgub[:, k * P:(k + 1) * P],
                                         rhs=xt_t[k][:, n * 512:(n + 1) * 512],
                                         start=(k == 0), stop=(k == KH - 1))
                nc.scalar.activation(sg_t[mi][:], psgs[:], SILU, bias=zbias[:])
                psus = ps_pool.tile([P, T], f32, name=f"psus{mi}", tag="B1", bufs=2)
                for k in range(KH):
                    for n in range(NT):
                        nc.tensor.matmul(psus[:, n * 512:(n + 1) * 512],
                                         lhsT=sgub[:, (KH + k) * P:(KH + k + 1) * P],
                                         rhs=xt_t[k][:, n * 512:(n + 1) * 512],
                                         start=(k == 0), stop=(k == KH - 1))
                nc.vector.tensor_mul(out=as_t[mi][:], in0=sg_t[mi][:], in1=psus[:])

            def emit_shared_down(ms):
                for mg in ms:
                    sdb = sst_pool.tile([P, 4 * KS * P], bf16, name=f"sdb{mg}", tag="sdb", bufs=2)
                    nc.sync.dma_start(sdb[:], swd_h[mg * P:(mg + 1) * P, :])
                    for mh in range(4):
                        m = mg * 4 + mh
                        psys = ps_pool.tile([P, T], f32, name=f"psys{m}", tag="B1", bufs=2)
                        for ki in range(KS):
                            for n in range(NT):
                                nc.tensor.matmul(psys[:, n * 512:(n + 1) * 512],
                                                 lhsT=sdb[:, (mh * KS + ki) * P:(mh * KS + ki + 1) * P],
                                                 rhs=as_t[ki][:, n * 512:(n + 1) * 512],
                                                 start=(ki == 0), stop=(ki == KS - 1))
                        sstg = stage_pool.tile([P, T], bf16, name=f"sstg{m}", tag="sstage", bufs=2)
                        # alternate the PSUM eviction engine so neither
                        # scalar nor vector saturates during the tail
                        if m % 2 == 0:
                            nc.scalar.copy(sstg[:], psys[:])
                        else:
                            nc.vector.tensor_copy(sstg[:], psys[:])
                        nc.gpsimd.dma_start(st_h[m * P:(m + 1) * P, :], sstg[:])

            # PE-section order: start with upgate(0) (only needs xgb0 + the
            # first wgu tile, ~2.3MB); gate runs after it, by which time xt
            # has streamed in on the scalar queue.  PE-light shared sections
            # are interleaved into the DMA-heavy routed sections so the
            # weight-stream demand stays below the ~330GB/s HBM rate.
            def upg0_ilv(m):
                # stagger the xt resident loads through the m-loop, each
                # placed after a weight-DMA that waits on buffer reuse so
                # they trail the weight stream instead of racing it
                if m in (5, 7, 9):
                    load_xt((m - 5) // 2)

            emit_upgate(0, interleave=upg0_ilv)
            load_xt(3)
            nc.sync.dma_start(xgb[1][:], xg_h[1][:])
            emit_gate()
            emit_shared_ug(0)
            emit_gather(0)
            emit_gather(1)
            emit_down(0, interleave=lambda mg: emit_shared_ug(1) if mg == 2 else None)
            emit_upgate(1, interleave=lambda m: emit_shared_ug(2) if m == 5 else None)
            emit_down(1, interleave=lambda mg: emit_shared_down([mg]) if mg < 4 else None)

    nc.compile()
    return nc


def _get_nc(CA, CB):
    if (CA, CB) not in _NC_CACHE:
        _NC_CACHE[(CA, CB)] = _build(CA, CB)
    return _NC_CACHE[(CA, CB)]


def kernel(**inputs):
    global LAST_RESULTS
    from concourse.bass_utils import run_bass_kernel_spmd

    BF16 = _bf16()

    hs = np.asarray(inputs["hidden_states"], dtype=np.float32)
    gate_w = np.asarray(inputs["gate_w"], dtype=np.float32)
    w_gate = np.asarray(inputs["w_gate"], dtype=np.float32)
    w_up = np.asarray(inputs["w_up"], dtype=np.float32)
    w_down = np.asarray(inputs["w_down"], dtype=np.float32)
    sw_gate = np.asarray(inputs["sw_gate"], dtype=np.float32)
    sw_up = np.asarray(inputs["sw_up"], dtype=np.float32)
    sw_down = np.asarray(inputs["sw_down"], dtype=np.float32)

    orig_shape = hs.shape
    x = hs.reshape(-1, H)
    assert x.shape[0] == T

    # ---- host: discrete routing only (top-4 selection + dispatch tables) ----
    logits = x @ gate_w.T
    smax = logits.max(axis=-1, keepdims=True)
    sc = np.exp(logits - smax)
    sc /= sc.sum(axis=-1, keepdims=True)
    order = np.argsort(-sc, axis=-1, kind="stable")[:, :TOPK]
    mask = np.zeros((T, E), dtype=np.float32)
    mask[np.arange(T)[:, None], order] = 1.0
    tok_lists = [np.nonzero(mask[:, e])[0].astype(np.int64) for e in range(E)]

    # balance: pair the i-th most-loaded expert with the i-th least-loaded
    sizes = np.array([len(tk) for tk in tok_lists])
    by_load = np.argsort(-sizes, kind="stable")
    pairs = [(int(by_load[i]), int(by_load[E - 1 - i])) for i in range(NCORES)]
    CA = max(64, int(np.ceil(max(sizes[p[0]] for p in pairs) / 32)) * 32)
    CB = max(64, int(np.ceil(max(sizes[p[1]] for p in pairs) / 32)) * 32)
    CJ = [CA, CB]

    nc = _get_nc(CA, CB)

    xT = np.ascontiguousarray(x.T)
    xTb = xT.astype(BF16)
    # xt packed for 4-ktile row loads: xtp[kk*P + p, a*T + t] = x[t, (4kk+a)*P + p]
    xtp = _group_rows(xTb, KH, T, 4)
    # gate weights packed: gwtb[p, k*E + e] = gate_w[e, k*P + p]
    gwtb = np.ascontiguousarray(
        gate_w.T.reshape(KH, P, E).transpose(1, 0, 2).reshape(P, KH * E)).astype(BF16)
    # mask packed: maskb[p, t8*E + e] = mask[t8*P + p, e]
    maskb = np.ascontiguousarray(mask.reshape(T // P, P, E).transpose(1, 0, 2).reshape(P, (T // P) * E))

    # shared slices, zero-padded to 384 and tile-major packed
    def pad_cols(w, newc):
        out = np.zeros((w.shape[0], newc), dtype=np.float32)
        out[:, :w.shape[1]] = w
        return out

    def pad_rows(w, newr):
        out = np.zeros((newr, w.shape[1]), dtype=np.float32)
        out[:w.shape[0], :] = w
        return out

    in_maps = []
    for c in range(NCORES):
        es = pairs[c]
        widx = np.full((CA + CB, 1), ZERO_ROW_FLAT, dtype=np.int32)
        sg_p = _pack_st(pad_cols(sw_gate[:, c * ISS:(c + 1) * ISS], ISSP), KH, KS)
        su_p = _pack_st(pad_cols(sw_up[:, c * ISS:(c + 1) * ISS], ISSP), KH, KS)
        im = {
            "xt": xtp, "gwtb": gwtb, "maskb": maskb, "widx": widx,
            "swgu": _concat_cols([sg_p, su_p], KS, KH * P).astype(BF16),
            "swd": _group_rows(
                _pack_st(pad_rows(sw_down[c * ISS:(c + 1) * ISS, :], ISSP), KS, MH),
                MH, KS * P, 4).astype(BF16),
        }
        coff = 0
        for j, e in enumerate(es):
            tk = tok_lists[e]
            C = CJ[j]
            widx[coff:coff + len(tk), 0] = (tk * E + e).astype(np.int32)
            coff += C
            # gathered activations, tile-major: xg[p, k*C + c] = x[tok_c, k*P + p]
            xg = np.zeros((P, KH * C), dtype=BF16)
            g = xTb[:, tk].reshape(KH, P, len(tk)).transpose(1, 0, 2)  # [P, KH, n]
            xg.reshape(P, KH, C)[:, :, :len(tk)] = g
            im[f"xg{j}"] = xg
            wg_p = _pack_st(w_gate[e], KH, MI)
            wu_p = _pack_st(w_up[e], KH, MI)
            im[f"wgu{j}"] = _concat_cols([wg_p, wu_p], MI, KH * P).astype(BF16)
            im[f"wd{j}"] = _group_rows(_pack_st(w_down[e], KI, MH), MH, KI * P, 2).astype(BF16)
        in_maps.append(im)

    trace = bool(int(os.environ.get("BASSMOE_TRACE", "0")))
    kwargs = {}
    if trace:
        kwargs = dict(trace=True, tmpdir=os.environ.get("BASSMOE_TRACE_DIR") or None)
        tcores = os.environ.get("BASSMOE_TRACE_CORES")
        if tcores:
            kwargs["trace_cores"] = [int(x) for x in tcores.split(",")]
            kwargs["stitch_traces"] = False
    res = run_bass_kernel_spmd(nc, in_maps, core_ids=list(range(NCORES)), **kwargs)
    LAST_RESULTS = res

    # ---- host: unshard (scatter-add compact expert outputs + sum partials) ----
    y = np.zeros((T, H), dtype=np.float64)
    st_sum = np.zeros((H, T), dtype=np.float64)
    for c in range(NCORES):
        r = res.results[c]
        st_sum += np.asarray(r["st"], dtype=np.float64)
        coff = 0
        for j, e in enumerate(pairs[c]):
            tk = tok_lists[e]
            zt = np.asarray(r["zt"], dtype=np.float64)
            y[tk] += zt[:, coff:coff + len(tk)].T
            coff += CJ[j]
    y += st_sum.T
    return y.astype(np.float32).reshape(orig_shape)


# revision 31
# speedup vs baseline: 1.1332x; 1.1086x over previous
"""DeepseekMoE layer on 8 Trainium2 NeuronCores (Bass/Tile, expert-parallel).

Sharding (per the expert-parallel hint):
  - 16 routed experts -> 2 per core, paired big+small by token count so the
    per-core slot totals balance; token dispatch (all-to-all) is emulated at
    the sharding layer: the host computes the discrete top-4 routing, gathers
    each expert's tokens into a compact transposed batch, and scatter-adds the
    compact expert outputs back into the full output ("combine").
  - Shared expert is tensor-parallel over its intermediate dim (2816/8 = 352
    columns per core, padded to 384); the 8 partial outputs are summed on
    gather.
  - Gate (softmax + renormalized top-4 combine weights) is replicated and
    computed ON DEVICE from the hidden states; the host only supplies the
    discrete 0/1 top-4 mask (routing decision) and gather indices.

All FLOPs that produce output values run on device.  Matmul operands are
bf16 (fp32 PSUM accumulation): on TRN2 the PE runs bf16 and fp32r at the
same 1 row/cycle, so bf16 costs no compute but halves the HBM weight/
activation traffic, which is what bounds this kernel.  Only the tiny gate
softmax pipeline stays fp32.

Weights are host-packed into stationary-tile-major layout ([m-tile,
partition, k-tile, col]); gate+up (and consecutive down-proj m-tiles) are
further interleaved so one DMA instruction streams 0.7-1MB with 4KB
descriptors, keeping the DMA issue queues short.
"""

import os
import numpy as np

H = 2048          # hidden size
E = 16            # routed experts
TOPK = 4
I = 1408          # routed expert intermediate
ISH = 2816        # shared expert intermediate
T = 1024          # tokens
P = 128
NCORES = 8
EPC = 2           # experts per core
ISS = ISH // NCORES                  # 352 shared columns per core
ISSP = 384                           # padded to 3 full 128-tiles
KH = H // P                          # 16 k-tiles over H
MI = I // P                          # 11 m-tiles over I
MH = H // P                          # 16 m-tiles over H
KI = I // P                          # 11 k-tiles over I
KS = ISSP // P                       # 3 k-tiles over padded shared slice
ZERO_ROW_FLAT = T * E                # flat index of the zeroed scratch row

_NC_CACHE = {}
LAST_RESULTS = None  # BassKernelResults of the most recent run (for test.py)


def _bf16():
    import ml_dtypes
    return ml_dtypes.bfloat16


def _token_chunks(C):
    """Split [0, C) into matmul moving-dim chunks of <=512."""
    out = []
    off = 0
    while off < C:
        sz = min(512, C - off)
        out.append((off, sz))
        off += sz
    return out


def _pack_st(w, KT, MT):
    """[KT*P, MT*P] -> [MT*P, KT*P] tile-major stationary pack.

    packed[m*P + p, k*P + c] = w[k*P + p, m*P + c], so the device loads
    rows [m*P, (m+1)*P) as one [P, KT*P] block whose column-slice k is the
    stationary tile for (k, m).
    """
    return np.ascontiguousarray(
        w.reshape(KT, P, MT, P).transpose(2, 1, 0, 3).reshape(MT * P, KT * P))


def _concat_cols(packs, MT, KTP):
    """n [MT*P, KTP] packs -> [MT*P, n*KTP]: row (m*P+p) = concat of each pack's
    (m,p) row, so one contiguous-row DMA (hardware DGE, no rearrange) streams
    all n stationary blocks for m-tile m."""
    return np.ascontiguousarray(
        np.concatenate([w.reshape(MT, P, KTP) for w in packs], axis=2)
        .reshape(MT * P, len(packs) * KTP))


def _group_rows(w, MT, KTP, G):
    """[MT*P, KTP] -> [(MT//G)*P, G*KTP]: row (g*P+p) = concat of m-tiles
    g*G..g*G+G-1's row p, so G m-tiles load as one contiguous-row DMA."""
    return _concat_cols([w.reshape(MT, P, KTP)[i::G].reshape(MT // G * P, KTP)
                         for i in range(G)], MT // G, KTP)


def _build(CA, CB):
    import concourse.bacc as bacc
    import concourse.bass as bass
    import concourse.mybir as mybir
    import concourse.tile as tile
    from concourse.masks import make_identity

    f32 = mybir.dt.float32
    bf16 = mybir.dt.bfloat16
    i32 = mybir.dt.int32
    SILU = mybir.ActivationFunctionType.Silu
    EXP = mybir.ActivationFunctionType.Exp
    X = mybir.AxisListType.X

    CJ = [CA, CB]
    CHJ = [_token_chunks(CA), _token_chunks(CB)]
    COFFJ = [0, CA]        # column offsets in zt
    CT = CA + CB
    NT = T // 512     # token chunks for shared/gate (2)
    NCHJ = [(CA + P - 1) // P, (CB + P - 1) // P]   # P-chunks per expert
    NCH = NCHJ[0] + NCHJ[1]
    T8 = T // P       # 8 token-tiles for the gate

    nc = bacc.Bacc("TRN2", target_bir_lowering=False, debug=False)

    xt_h = nc.dram_tensor("xt", [(KH // 4) * P, 4 * T], bf16, kind="ExternalInput")
    gwtb_h = nc.dram_tensor("gwtb", [P, KH * E], bf16, kind="ExternalInput")
    maskb_h = nc.dram_tensor("maskb", [P, (T // P) * E], f32, kind="ExternalInput")
    xg_h = [nc.dram_tensor(f"xg{j}", [P, KH * CJ[j]], bf16, kind="ExternalInput") for j in range(EPC)]
    widx_h = nc.dram_tensor("widx", [P, NCH], i32, kind="ExternalInput")
    wgu_h = [nc.dram_tensor(f"wgu{j}", [MI * P, 2 * KH * P], bf16, kind="ExternalInput") for j in range(EPC)]
    wd_h = [nc.dram_tensor(f"wd{j}", [(MH // 2) * P, 2 * KI * P], bf16, kind="ExternalInput") for j in range(EPC)]
    swgu_h = nc.dram_tensor("swgu", [KS * P, 2 * KH * P], bf16, kind="ExternalInput")
    swd_h = nc.dram_tensor("swd", [(MH // 4) * P, 4 * KS * P], bf16, kind="ExternalInput")
    zt_h = nc.dram_tensor("zt", [H, CT], bf16, kind="ExternalOutput")
    st_h = nc.dram_tensor("st", [H, T], bf16, kind="ExternalOutput")

    with tile.TileContext(nc) as tc:
        with (
            tc.tile_pool(name="resident", bufs=1) as res_pool,
            tc.tile_pool(name="xgp", bufs=1) as xg_pool,
            tc.tile_pool(name="acts", bufs=1) as act_pool,
            tc.tile_pool(name="wstream", bufs=3) as wst_pool,
            tc.tile_pool(name="dstream", bufs=3) as dst_pool,
            tc.tile_pool(name="sstream", bufs=2) as sst_pool,
            tc.tile_pool(name="small", bufs=2) as small_pool,
            tc.tile_pool(name="stage", bufs=3) as stage_pool,
            tc.tile_pool(name="ps", bufs=1, space="PSUM") as ps_pool,
            tc.tile_pool(name="dram", bufs=1, space="DRAM") as dram_pool,
        ):
            # ---------------- resident loads ----------------
            # xgb0 (gpsimd queue) + first wgu tiles (sync queue) land first so
            # upgate(0) starts the PE ~5us in; xt and xgb1 are issued from
            # inside the upgate(0) m-loop so they don't steal HBM bandwidth
            # from the weight stream during startup (gate needs xt only ~50us
            # in, upgate(1) needs xgb1 ~130us in).
            xgb = [xg_pool.tile([P, KH * CJ[j]], bf16, name=f"xgb{j}", tag="xgb") for j in range(EPC)]
            nc.gpsimd.dma_start(xgb[0][:], xg_h[0][:])
            xt4 = [res_pool.tile([P, 4 * T], bf16, name=f"xt4_{kk}", tag=f"xt4_{kk}") for kk in range(KH // 4)]

            def load_xt(kk):
                # on the sync queue: DMA descriptors drain FIFO across all
                # issuing engines, so xt must enqueue BEHIND the wgu weight
                # tiles (whose issues are throttled by pool-buffer reuse) or
                # it steals the weight stream's HBM bandwidth at startup
                nc.sync.dma_start(xt4[kk][:], xt_h[kk * P:(kk + 1) * P, :])

            xt_t = [xt4[k // 4][:, (k % 4) * T:(k % 4 + 1) * T] for k in range(KH)]
            gwtb = res_pool.tile([P, KH * E], bf16, name="gwtb", tag="gwtb")
            nc.gpsimd.dma_start(gwtb[:], gwtb_h[:])
            maskb = res_pool.tile([P, (T // P) * E], f32, name="maskb", tag="maskb")
            nc.gpsimd.dma_start(maskb[:], maskb_h[:])
            ident = res_pool.tile([P, P], f32, name="ident", tag="ident")
            make_identity(nc, ident[:])
            zbias = res_pool.tile([P, 1], f32, name="zbias", tag="zbias")
            nc.vector.memset(zbias[:], 0.0)
            # combine-weight gather indices: no data deps, preloaded at start
            it_all = res_pool.tile([P, NCH], i32, name="it_all", tag="it_all")
            nc.gpsimd.dma_start(it_all[:], widx_h[:])

            # combine-weight scratch in HBM: rows 0..T-1 = combine, row T = zeros
            wflat = dram_pool.tile([(T + 1) * E, 1], f32, name="wflat")
            wflat2d = wflat[:].rearrange("(a b) o -> a (b o)", b=E)
            zrow = res_pool.tile([1, E], f32, name="zrow", tag="zrow")
            nc.vector.memset(zrow[:], 0.0)
            nc.gpsimd.dma_start(wflat2d[T:T + 1, :], zrow[:])

            wb = [res_pool.tile([P, CJ[j]], f32, name=f"wb{j}", tag=f"wb{j}") for j in range(EPC)]
            a_t = [[act_pool.tile([P, CJ[j]], bf16, name=f"a{j}_{m}", tag=f"a{j}_{m}") for m in range(MI)]
                   for j in range(EPC)]
            sg_t = [act_pool.tile([P, T], f32, name=f"sg{m}", tag="sgtmp", bufs=2) for m in range(KS)]
            as_t = [act_pool.tile([P, T], bf16, name=f"as{m}", tag=f"as{m}") for m in range(KS)]

            # ---------------- emission sections ----------------
            def emit_gate():
                lgps = ps_pool.tile([E, T], f32, name="lgps", tag="B1", bufs=2)
                for n in range(NT):
                    for k in range(KH):
                        nc.tensor.matmul(
                            lgps[:, n * 512:(n + 1) * 512],
                            lhsT=gwtb[:, k * E:(k + 1) * E],
                            rhs=xt_t[k][:, n * 512:(n + 1) * 512],
                            start=(k == 0), stop=(k == KH - 1),
                        )
                # batched softmax over all 8 token-tiles: one transpose batch
                # into PSUM [P, 8E=128], then single-instruction exp/mask/
                # reduce/normalize so the combine-weight chain is short
                lgsb = res_pool.tile([E, T], f32, name="lgsb", tag="lgsb")
                nc.scalar.copy(lgsb[:], lgps[:])
                trps = ps_pool.tile([P, T8 * E], f32, name="trall", tag="A1", bufs=4)
                for t8 in range(T8):
                    nc.tensor.transpose(
                        out=trps[:, t8 * E:(t8 + 1) * E],
                        in_=lgsb[:, t8 * P:(t8 + 1) * P], identity=ident[0:E, 0:E],
                    )
                sc = small_pool.tile([P, T8 * E], f32, name="sc", tag="sc")
                nc.scalar.activation(sc[:], trps[:], EXP, bias=zbias[:])
                mskd = small_pool.tile([P, T8 * E], f32, name="mskd", tag="mskd")
                nc.vector.tensor_mul(out=mskd[:], in0=sc[:], in1=maskb[:])
                ssum = small_pool.tile([P, T8], f32, name="ssum", tag="ssum")
                nc.vector.reduce_sum(
                    ssum[:], mskd[:].rearrange("p (a e) -> p a e", e=E), axis=X)
                rsum = small_pool.tile([P, T8], f32, name="rsum", tag="rsum")
                nc.vector.reciprocal(rsum[:], ssum[:])
                comb = small_pool.tile([P, T8 * E], f32, name="comb", tag="comb")
                for t8 in range(T8):
                    nc.vector.tensor_scalar_mul(
                        comb[:, t8 * E:(t8 + 1) * E],
                        mskd[:, t8 * E:(t8 + 1) * E], rsum[:, t8:t8 + 1])
                nc.gpsimd.dma_start(
                    wflat2d[0:T, :].rearrange("(a p) e -> p a e", p=P),
                    comb[:].rearrange("p (a e) -> p a e", e=E))

            def emit_gather():
                # one indirect gather for every P-chunk of both experts, then
                # per-chunk PE transposes to partition-broadcast wb[j]
                wsall = small_pool.tile([P, NCH], f32, name="wsall", tag="ws")
                for c0 in range(NCH):
                    nc.gpsimd.indirect_dma_start(
                        out=wsall[:, c0:c0 + 1], out_offset=None, in_=wflat[:],
                        in_offset=bass.IndirectOffsetOnAxis(ap=it_all[:, c0:c0 + 1], axis=0),
                    )
                cc = 0
                for j in range(EPC):
                    for i in range(NCHJ[j]):
                        off = i * P
                        csz = min(P, CJ[j] - off)
                        wbps = ps_pool.tile([P, P], f32, name=f"wbps{j}_{off}", tag="A1", bufs=4)
                        nc.tensor.transpose(
                            out=wbps[:, :csz],
                            in_=wsall[:csz, cc:cc + 1].to_broadcast([csz, P]),
                            identity=ident[0:csz, 0:csz],
                        )
                        nc.vector.tensor_copy(wb[j][:, off:off + csz], wbps[:, :csz])
                        cc += 1

            def emit_upgate(j, interleave=None):
                xg_t = [xgb[j][:, k * CJ[j]:(k + 1) * CJ[j]] for k in range(KH)]
                g_t = [act_pool.tile([P, CJ[j]], f32, name=f"g{j}_{m}", tag="gtmp", bufs=3) for m in range(MI)]
                for m in range(MI):
                    if interleave is not None:
                        interleave(m)
                    wgub = wst_pool.tile([P, 2 * KH * P], bf16, name=f"wgub{j}_{m}", tag="wblk", bufs=4)
                    nc.sync.dma_start(wgub[:], wgu_h[j][m * P:(m + 1) * P, :])
                    wgb = [wgub[:, k * P:(k + 1) * P] for k in range(KH)]
                    wub = [wgub[:, (KH + k) * P:(KH + k + 1) * P] for k in range(KH)]
                    for (coff, csz) in CHJ[j]:
                        psg = ps_pool.tile([P, csz], f32, name=f"psg{j}_{m}_{coff}", tag="A1", bufs=4)
                        for k in range(KH):
                            nc.tensor.matmul(psg[:], lhsT=wgb[k],
                                             rhs=xg_t[k][:, coff:coff + csz],
                                             start=(k == 0), stop=(k == KH - 1))
                        nc.scalar.activation(g_t[m][:, coff:coff + csz], psg[:], SILU, bias=zbias[:])
                        psu = ps_pool.tile([P, csz], f32, name=f"psu{j}_{m}_{coff}", tag="A1", bufs=4)
                        for k in range(KH):
                            nc.tensor.matmul(psu[:], lhsT=wub[k],
                                             rhs=xg_t[k][:, coff:coff + csz],
                                             start=(k == 0), stop=(k == KH - 1))
                        # a = silu(g) * u straight out of PSUM, rounded to bf16
                        nc.vector.tensor_mul(out=a_t[j][m][:, coff:coff + csz],
                                             in0=g_t[m][:, coff:coff + csz], in1=psu[:])

            def emit_down(j, interleave=None):
                for mg in range(MH // 2):
                    if interleave is not None:
                        interleave(mg)
                    wdb = dst_pool.tile([P, 2 * KI * P], bf16, name=f"wdb{j}_{mg}", tag="wdb", bufs=4)
                    nc.sync.dma_start(wdb[:], wd_h[j][mg * P:(mg + 1) * P, :])
                    # the final groups' writes go via sync's hardware DGE so
                    # the end-of-kernel gpsimd drain isn't waiting on SWDGE
                    weng = nc.sync if (j == 1 and mg >= MH // 2 - 2) else nc.gpsimd
                    for mh in range(2):
                        m = mg * 2 + mh
                        for (coff, csz) in CHJ[j]:
                            psz = ps_pool.tile([P, csz], f32, name=f"psz{j}_{m}_{coff}", tag="A1", bufs=4)
                            for k in range(KI):
                                nc.tensor.matmul(psz[:], lhsT=wdb[:, (mh * KI + k) * P:(mh * KI + k + 1) * P],
                                                 rhs=a_t[j][k][:, coff:coff + csz],
                                                 start=(k == 0), stop=(k == KI - 1))
                            zst = stage_pool.tile([P, csz], bf16, name=f"zst{j}_{m}_{coff}", tag="zst", bufs=2)
                            # combine-weight scaling fused into the eviction
                            nc.vector.tensor_mul(out=zst[:], in0=wb[j][:, coff:coff + csz], in1=psz[:])
                            weng.dma_start(
                                zt_h[m * P:(m + 1) * P, COFFJ[j] + coff:COFFJ[j] + coff + csz], zst[:])

            def emit_shared_ug(mi):
                sgub = sst_pool.tile([P, 2 * KH * P], bf16, name=f"sgub{mi}", tag="ssb", bufs=3)
                nc.sync.dma_start(sgub[:], swgu_h[mi * P:(mi + 1) * P, :])
                psgs = ps_pool.tile([P, T], f32, name=f"psgs{mi}", tag="B1", bufs=2)
                for k in range(KH):
                    for n in range(NT):
                        nc.tensor.matmul(psgs[:, n * 512:(n + 1) * 512],
                                         lhsT=sgub[:, k * P:(k + 1) * P],
                                         rhs=xt_t[k][:, n * 512:(n + 1) * 512],
                                         start=(k == 0), stop=(k == KH - 1))
                nc.scalar.activation(sg_t[mi][:], psgs[:], SILU, bias=zbias[:])
                psus = ps_pool.tile([P, T], f32, name=f"psus{mi}", tag="B1", bufs=2)
                for k in range(KH):
                    for n in range(NT):
                        nc.tensor.matmul(psus[:, n * 512:(n + 1) * 512],
                                         lhsT=sgub[:, (KH + k) * P:(KH + k + 1) * P],
                                         rhs=xt_t[k][:, n * 512:(n + 1) * 512],
                                         start=(k == 0), stop=(k == KH - 1))
                nc.vector.tensor_mul(out=as_t[mi][:], in0=sg_t[mi][:], in1=psus[:])

            def emit_shared_down(ms):
                for mg in ms:
                    sdb = sst_pool.tile([P, 4 * KS * P], bf16, name=f"sdb{mg}", tag="sdb", bufs=2)
                    nc.sync.dma_start(sdb[:], swd_h[mg * P:(mg + 1) * P, :])
                    for mh in range(4):
                        m = mg * 4 + mh
                        psys = ps_pool.tile([P, T], f32, name=f"psys{m}", tag="B1", bufs=2)
                        for ki in range(KS):
                            for n in range(NT):
                                nc.tensor.matmul(psys[:, n * 512:(n + 1) * 512],
                                                 lhsT=sdb[:, (mh * KS + ki) * P:(mh * KS + ki + 1) * P],
                                                 rhs=as_t[ki][:, n * 512:(n + 1) * 512],
                                                 start=(ki == 0), stop=(ki == KS - 1))
                        sstg = stage_pool.tile([P, T], bf16, name=f"sstg{m}", tag="sstage", bufs=2)
                        # alternate the PSUM eviction engine so neither
                        # scalar nor vector saturates during the tail
                        if m % 2 == 0:
                            nc.scalar.copy(sstg[:], psys[:])
                        else:
                            nc.vector.tensor_copy(sstg[:], psys[:])
                        nc.sync.dma_start(st_h[m * P:(m + 1) * P, :], sstg[:])

            # PE-section order: start with upgate(0) (only needs xgb0 + the
            # first wgu tile, ~2.3MB); gate runs after it, by which time xt
            # has streamed in on the scalar queue.  PE-light shared sections
            # are interleaved into the DMA-heavy routed sections so the
            # weight-stream demand stays below the ~330GB/s HBM rate.
            def upg0_ilv(m):
                # stagger the xt resident loads through the m-loop, each
                # placed after a weight-DMA that waits on buffer reuse so
                # they trail the weight stream instead of racing it
                if m in (5, 7, 9):
                    load_xt((m - 5) // 2)

            emit_upgate(0, interleave=upg0_ilv)
            load_xt(3)
            nc.sync.dma_start(xgb[1][:], xg_h[1][:])
            emit_gate()
            emit_shared_ug(0)
            emit_gather()
            emit_down(0, interleave=lambda mg: emit_shared_ug(1) if mg == 2 else None)
            emit_upgate(1, interleave=lambda m: emit_shared_ug(2) if m == 5 else None)
            emit_down(1, interleave=lambda mg: emit_shared_down([mg]) if mg < 4 else None)

    nc.compile()
    return nc


def _get_nc(CA, CB):
    if (CA, CB) not in _NC_CACHE:
        _NC_CACHE[(CA, CB)] = _build(CA, CB)
    return _NC_CACHE[(CA, CB)]


def kernel(**inputs):
    global LAST_RESULTS
    from concourse.bass_utils import run_bass_kernel_spmd

    BF16 = _bf16()

    hs = np.asarray(inputs["hidden_states"], dtype=np.float32)
    gate_w = np.asarray(inputs["gate_w"], dtype=np.float32)
    w_gate = np.asarray(inputs["w_gate"], dtype=np.float32)
    w_up = np.asarray(inputs["w_up"], dtype=np.float32)
    w_down = np.asarray(inputs["w_down"], dtype=np.float32)
    sw_gate = np.asarray(inputs["sw_gate"], dtype=np.float32)
    sw_up = np.asarray(inputs["sw_up"], dtype=np.float32)
    sw_down = np.asarray(inputs["sw_down"], dtype=np.float32)

    orig_shape = hs.shape
    x = hs.reshape(-1, H)
    assert x.shape[0] == T

    # ---- host: discrete routing only (top-4 selection + dispatch tables) ----
    logits = x @ gate_w.T
    smax = logits.max(axis=-1, keepdims=True)
    sc = np.exp(logits - smax)
    sc /= sc.sum(axis=-1, keepdims=True)
    order = np.argsort(-sc, axis=-1, kind="stable")[:, :TOPK]
    mask = np.zeros((T, E), dtype=np.float32)
    mask[np.arange(T)[:, None], order] = 1.0
    tok_lists = [np.nonzero(mask[:, e])[0].astype(np.int64) for e in range(E)]

    # balance: pair the i-th most-loaded expert with the i-th least-loaded
    sizes = np.array([len(tk) for tk in tok_lists])
    by_load = np.argsort(-sizes, kind="stable")
    pairs = [(int(by_load[i]), int(by_load[E - 1 - i])) for i in range(NCORES)]
    CA = max(64, int(np.ceil(max(sizes[p[0]] for p in pairs) / 32)) * 32)
    CB = max(64, int(np.ceil(max(sizes[p[1]] for p in pairs) / 32)) * 32)
    CJ = [CA, CB]

    nc = _get_nc(CA, CB)

    xT = np.ascontiguousarray(x.T)
    xTb = xT.astype(BF16)
    # xt packed for 4-ktile row loads: xtp[kk*P + p, a*T + t] = x[t, (4kk+a)*P + p]
    xtp = _group_rows(xTb, KH, T, 4)
    # gate weights packed: gwtb[p, k*E + e] = gate_w[e, k*P + p]
    gwtb = np.ascontiguousarray(
        gate_w.T.reshape(KH, P, E).transpose(1, 0, 2).reshape(P, KH * E)).astype(BF16)
    # mask packed: maskb[p, t8*E + e] = mask[t8*P + p, e]
    maskb = np.ascontiguousarray(mask.reshape(T // P, P, E).transpose(1, 0, 2).reshape(P, (T // P) * E))

    # shared slices, zero-padded to 384 and tile-major packed
    def pad_cols(w, newc):
        out = np.zeros((w.shape[0], newc), dtype=np.float32)
        out[:, :w.shape[1]] = w
        return out

    def pad_rows(w, newr):
        out = np.zeros((newr, w.shape[1]), dtype=np.float32)
        out[:w.shape[0], :] = w
        return out

    nchA = (CA + P - 1) // P
    nchB = (CB + P - 1) // P
    in_maps = []
    for c in range(NCORES):
        es = pairs[c]
        # widx[p, cc] = flat wflat row for slot cc*P + p of the concatenated
        # (expert A chunks, expert B chunks) compact batch
        widx = np.full((P, nchA + nchB), ZERO_ROW_FLAT, dtype=np.int32)
        sg_p = _pack_st(pad_cols(sw_gate[:, c * ISS:(c + 1) * ISS], ISSP), KH, KS)
        su_p = _pack_st(pad_cols(sw_up[:, c * ISS:(c + 1) * ISS], ISSP), KH, KS)
        im = {
            "xt": xtp, "gwtb": gwtb, "maskb": maskb, "widx": widx,
            "swgu": _concat_cols([sg_p, su_p], KS, KH * P).astype(BF16),
            "swd": _group_rows(
                _pack_st(pad_rows(sw_down[c * ISS:(c + 1) * ISS, :], ISSP), KS, MH),
                MH, KS * P, 4).astype(BF16),
        }
        for j, e in enumerate(es):
            tk = tok_lists[e]
            C = CJ[j]
            ccoff = 0 if j == 0 else nchA
            flat = (tk * E + e).astype(np.int32)
            for s in range(len(tk)):
                widx[s % P, ccoff + s // P] = flat[s]
            # gathered activations, tile-major: xg[p, k*C + c] = x[tok_c, k*P + p]
            xg = np.zeros((P, KH * C), dtype=BF16)
            g = xTb[:, tk].reshape(KH, P, len(tk)).transpose(1, 0, 2)  # [P, KH, n]
            xg.reshape(P, KH, C)[:, :, :len(tk)] = g
            im[f"xg{j}"] = xg
            wg_p = _pack_st(w_gate[e], KH, MI)
            wu_p = _pack_st(w_up[e], KH, MI)
            im[f"wgu{j}"] = _concat_cols([wg_p, wu_p], MI, KH * P).astype(BF16)
            im[f"wd{j}"] = _group_rows(_pack_st(w_down[e], KI, MH), MH, KI * P, 2).astype(BF16)
        in_maps.append(im)

    trace = bool(int(os.environ.get("BASSMOE_TRACE", "0")))
    kwargs = {}
    if trace:
        kwargs = dict(trace=True, tmpdir=os.environ.get("BASSMOE_TRACE_DIR") or None)
        tcores = os.environ.get("BASSMOE_TRACE_CORES")
        if tcores:
            kwargs["trace_cores"] = [int(x) for x in tcores.split(",")]
            kwargs["stitch_traces"] = False
    res = run_bass_kernel_spmd(nc, in_maps, core_ids=list(range(NCORES)), **kwargs)
    LAST_RESULTS = res

    # ---- host: unshard (scatter-add compact expert outputs + sum partials) ----
    y = np.zeros((T, H), dtype=np.float64)
    st_sum = np.zeros((H, T), dtype=np.float64)
    for c in range(NCORES):
        r = res.results[c]
        st_sum += np.asarray(r["st"], dtype=np.float64)
        coff = 0
        for j, e in enumerate(pairs[c]):
            tk = tok_lists[e]
            zt = np.asarray(r["zt"], dtype=np.float64)
            y[tk] += zt[:, coff:coff + len(tk)].T
            coff += CJ[j]
    y += st_sum.T
    return y.astype(np.float32).reshape(orig_shape)


# revision 36
# speedup vs baseline: 1.1657x; 1.0286x over previous
"""DeepseekMoE layer on 8 Trainium2 NeuronCores (Bass/Tile, expert-parallel).

Sharding (per the expert-parallel hint):
  - 16 routed experts -> 2 per core, paired big+small by token count so the
    per-core slot totals balance; token dispatch (all-to-all) is emulated at
    the sharding layer: the host computes the discrete top-4 routing, gathers
    each expert's tokens into a compact transposed batch, and scatter-adds the
    compact expert outputs back into the full output ("combine").
  - Shared expert is tensor-parallel over its intermediate dim (2816/8 = 352
    columns per core, padded to 384); the 8 partial outputs are summed on
    gather.
  - Gate (softmax + renormalized top-4 combine weights) is replicated and
    computed ON DEVICE from the hidden states; the host only supplies the
    discrete 0/1 top-4 mask (routing decision) and gather indices.

All FLOPs that produce output values run on device.  Matmul operands are
bf16 (fp32 PSUM accumulation): on TRN2 the PE runs bf16 and fp32r at the
same 1 row/cycle, so bf16 costs no compute but halves the HBM weight/
activation traffic, which is what bounds this kernel.  Only the tiny gate
softmax pipeline stays fp32.

Weights are host-packed into stationary-tile-major layout ([m-tile,
partition, k-tile, col]); gate+up (and consecutive down-proj m-tiles) are
further interleaved so one DMA instruction streams 0.7-1MB with 4KB
descriptors, keeping the DMA issue queues short.
"""

import os
import numpy as np

H = 2048          # hidden size
E = 16            # routed experts
TOPK = 4
I = 1408          # routed expert intermediate
ISH = 2816        # shared expert intermediate
T = 1024          # tokens
P = 128
NCORES = 8
EPC = 2           # experts per core
ISS = ISH // NCORES                  # 352 shared columns per core
ISSP = 384                           # padded to 3 full 128-tiles
KH = H // P                          # 16 k-tiles over H
MI = I // P                          # 11 m-tiles over I
MH = H // P                          # 16 m-tiles over H
KI = I // P                          # 11 k-tiles over I
KS = ISSP // P                       # 3 k-tiles over padded shared slice
ZERO_ROW_FLAT = T * E                # flat index of the zeroed scratch row

_NC_CACHE = {}
LAST_RESULTS = None  # BassKernelResults of the most recent run (for test.py)


def _bf16():
    import ml_dtypes
    return ml_dtypes.bfloat16


def _token_chunks(C):
    """Split [0, C) into matmul moving-dim chunks of <=512."""
    out = []
    off = 0
    while off < C:
        sz = min(512, C - off)
        out.append((off, sz))
        off += sz
    return out


def _pack_st(w, KT, MT):
    """[KT*P, MT*P] -> [MT*P, KT*P] tile-major stationary pack.

    packed[m*P + p, k*P + c] = w[k*P + p, m*P + c], so the device loads
    rows [m*P, (m+1)*P) as one [P, KT*P] block whose column-slice k is the
    stationary tile for (k, m).
    """
    return np.ascontiguousarray(
        w.reshape(KT, P, MT, P).transpose(2, 1, 0, 3).reshape(MT * P, KT * P))


def _concat_cols(packs, MT, KTP):
    """n [MT*P, KTP] packs -> [MT*P, n*KTP]: row (m*P+p) = concat of each pack's
    (m,p) row, so one contiguous-row DMA (hardware DGE, no rearrange) streams
    all n stationary blocks for m-tile m."""
    return np.ascontiguousarray(
        np.concatenate([w.reshape(MT, P, KTP) for w in packs], axis=2)
        .reshape(MT * P, len(packs) * KTP))


def _group_rows(w, MT, KTP, G):
    """[MT*P, KTP] -> [(MT//G)*P, G*KTP]: row (g*P+p) = concat of m-tiles
    g*G..g*G+G-1's row p, so G m-tiles load as one contiguous-row DMA."""
    return _concat_cols([w.reshape(MT, P, KTP)[i::G].reshape(MT // G * P, KTP)
                         for i in range(G)], MT // G, KTP)


def _build(CA, CB):
    import concourse.bacc as bacc
    import concourse.bass as bass
    import concourse.mybir as mybir
    import concourse.tile as tile
    from concourse.masks import make_identity

    f32 = mybir.dt.float32
    bf16 = mybir.dt.bfloat16
    i32 = mybir.dt.int32
    SILU = mybir.ActivationFunctionType.Silu
    EXP = mybir.ActivationFunctionType.Exp
    X = mybir.AxisListType.X

    CJ = [CA, CB]
    CHJ = [_token_chunks(CA), _token_chunks(CB)]
    COFFJ = [0, CA]        # column offsets in zt
    CT = CA + CB
    NT = T // 512     # token chunks for shared/gate (2)
    NCHJ = [(CA + P - 1) // P, (CB + P - 1) // P]   # P-chunks per expert
    NCH = NCHJ[0] + NCHJ[1]
    T8 = T // P       # 8 token-tiles for the gate

    nc = bacc.Bacc("TRN2", target_bir_lowering=False, debug=False)

    xt_h = nc.dram_tensor("xt", [(KH // 2) * P, 2 * T], bf16, kind="ExternalInput")
    gwtb_h = nc.dram_tensor("gwtb", [P, KH * E], bf16, kind="ExternalInput")
    maskb_h = nc.dram_tensor("maskb", [P, (T // P) * E], f32, kind="ExternalInput")
    xg_h = [nc.dram_tensor(f"xg{j}", [P, KH * CJ[j]], bf16, kind="ExternalInput") for j in range(EPC)]
    widx_h = nc.dram_tensor("widx", [P, NCH], i32, kind="ExternalInput")
    wgu_h = [nc.dram_tensor(f"wgu{j}", [MI * P, 2 * KH * P], bf16, kind="ExternalInput") for j in range(EPC)]
    wd_h = [nc.dram_tensor(f"wd{j}", [(MH // 2) * P, 2 * KI * P], bf16, kind="ExternalInput") for j in range(EPC)]
    swgu_h = nc.dram_tensor("swgu", [KS * P, 2 * KH * P], bf16, kind="ExternalInput")
    swd_h = nc.dram_tensor("swd", [(MH // 4) * P, 4 * KS * P], bf16, kind="ExternalInput")
    zt_h = nc.dram_tensor("zt", [H, CT], bf16, kind="ExternalOutput")
    st_h = nc.dram_tensor("st", [H, T], bf16, kind="ExternalOutput")

    with tile.TileContext(nc) as tc:
        with (
            tc.tile_pool(name="resident", bufs=1) as res_pool,
            tc.tile_pool(name="xgp", bufs=1) as xg_pool,
            tc.tile_pool(name="acts", bufs=1) as act_pool,
            tc.tile_pool(name="wstream", bufs=3) as wst_pool,
            tc.tile_pool(name="dstream", bufs=3) as dst_pool,
            tc.tile_pool(name="sstream", bufs=2) as sst_pool,
            tc.tile_pool(name="small", bufs=2) as small_pool,
            tc.tile_pool(name="stage", bufs=3) as stage_pool,
            tc.tile_pool(name="ps", bufs=1, space="PSUM") as ps_pool,
            tc.tile_pool(name="dram", bufs=1, space="DRAM") as dram_pool,
        ):
            # ---------------- resident loads ----------------
            # xgb0 (gpsimd queue) + first wgu tiles (sync queue) land first so
            # upgate(0) starts the PE ~5us in; xt and xgb1 are issued from
            # inside the upgate(0) m-loop so they don't steal HBM bandwidth
            # from the weight stream during startup (gate needs xt only ~50us
            # in, upgate(1) needs xgb1 ~130us in).
            xgb = [xg_pool.tile([P, KH * CJ[j]], bf16, name=f"xgb{j}", tag="xgb") for j in range(EPC)]
            xt2 = [res_pool.tile([P, 2 * T], bf16, name=f"xt2_{kk}", tag=f"xt2_{kk}") for kk in range(KH // 2)]

            def load_xt(kk):
                # DMA descriptors drain FIFO across all issuing engines, so
                # the issue order below is the bandwidth-allocation order
                nc.sync.dma_start(xt2[kk][:], xt_h[kk * P:(kk + 1) * P, :])

            xt_t = [xt2[k // 2][:, (k % 2) * T:(k % 2 + 1) * T] for k in range(KH)]
            gwtb = res_pool.tile([P, KH * E], bf16, name="gwtb", tag="gwtb")
            nc.gpsimd.dma_start(gwtb[:], gwtb_h[:])
            maskb = res_pool.tile([P, (T // P) * E], f32, name="maskb", tag="maskb")
            nc.gpsimd.dma_start(maskb[:], maskb_h[:])
            ident = res_pool.tile([P, P], f32, name="ident", tag="ident")
            make_identity(nc, ident[:])
            zbias = res_pool.tile([P, 1], f32, name="zbias", tag="zbias")
            nc.vector.memset(zbias[:], 0.0)
            # combine-weight gather indices: no data deps, preloaded at start
            it_all = res_pool.tile([P, NCH], i32, name="it_all", tag="it_all")
            nc.gpsimd.dma_start(it_all[:], widx_h[:])

            # combine-weight scratch in HBM: rows 0..T-1 = combine, row T = zeros
            wflat = dram_pool.tile([(T + 1) * E, 1], f32, name="wflat")
            wflat2d = wflat[:].rearrange("(a b) o -> a (b o)", b=E)
            zrow = res_pool.tile([1, E], f32, name="zrow", tag="zrow")
            nc.vector.memset(zrow[:], 0.0)
            nc.gpsimd.dma_start(wflat2d[T:T + 1, :], zrow[:])

            wb = [res_pool.tile([P, CJ[j]], f32, name=f"wb{j}", tag=f"wb{j}") for j in range(EPC)]
            a_t = [[act_pool.tile([P, CJ[j]], bf16, name=f"a{j}_{m}", tag=f"a{j}_{m}") for m in range(MI)]
                   for j in range(EPC)]
            sg_t = [act_pool.tile([P, T], f32, name=f"sg{m}", tag="sgtmp", bufs=2) for m in range(KS)]
            as_t = [act_pool.tile([P, T], bf16, name=f"as{m}", tag=f"as{m}") for m in range(KS)]

            # ---------------- emission sections ----------------
            def emit_gate():
                lgps = ps_pool.tile([E, T], f32, name="lgps", tag="B1", bufs=2)
                for n in range(NT):
                    for k in range(KH):
                        nc.tensor.matmul(
                            lgps[:, n * 512:(n + 1) * 512],
                            lhsT=gwtb[:, k * E:(k + 1) * E],
                            rhs=xt_t[k][:, n * 512:(n + 1) * 512],
                            start=(k == 0), stop=(k == KH - 1),
                        )
                # batched softmax over all 8 token-tiles: one transpose batch
                # into PSUM [P, 8E=128], then single-instruction exp/mask/
                # reduce/normalize so the combine-weight chain is short
                lgsb = res_pool.tile([E, T], f32, name="lgsb", tag="lgsb")
                nc.scalar.copy(lgsb[:], lgps[:])
                trps = ps_pool.tile([P, T8 * E], f32, name="trall", tag="A1", bufs=4)
                for t8 in range(T8):
                    nc.tensor.transpose(
                        out=trps[:, t8 * E:(t8 + 1) * E],
                        in_=lgsb[:, t8 * P:(t8 + 1) * P], identity=ident[0:E, 0:E],
                    )
                sc = small_pool.tile([P, T8 * E], f32, name="sc", tag="sc")
                nc.scalar.activation(sc[:], trps[:], EXP, bias=zbias[:])
                mskd = small_pool.tile([P, T8 * E], f32, name="mskd", tag="mskd")
                nc.vector.tensor_mul(out=mskd[:], in0=sc[:], in1=maskb[:])
                ssum = small_pool.tile([P, T8], f32, name="ssum", tag="ssum")
                nc.vector.reduce_sum(
                    ssum[:], mskd[:].rearrange("p (a e) -> p a e", e=E), axis=X)
                rsum = small_pool.tile([P, T8], f32, name="rsum", tag="rsum")
                nc.vector.reciprocal(rsum[:], ssum[:])
                comb = small_pool.tile([P, T8 * E], f32, name="comb", tag="comb")
                for t8 in range(T8):
                    nc.vector.tensor_scalar_mul(
                        comb[:, t8 * E:(t8 + 1) * E],
                        mskd[:, t8 * E:(t8 + 1) * E], rsum[:, t8:t8 + 1])
                nc.gpsimd.dma_start(
                    wflat2d[0:T, :].rearrange("(a p) e -> p a e", p=P),
                    comb[:].rearrange("p (a e) -> p a e", e=E))

            def emit_gather():
                # one indirect gather for every P-chunk of both experts, then
                # per-chunk PE transposes to partition-broadcast wb[j]
                wsall = small_pool.tile([P, NCH], f32, name="wsall", tag="ws")
                for c0 in range(NCH):
                    nc.gpsimd.indirect_dma_start(
                        out=wsall[:, c0:c0 + 1], out_offset=None, in_=wflat[:],
                        in_offset=bass.IndirectOffsetOnAxis(ap=it_all[:, c0:c0 + 1], axis=0),
                    )
                cc = 0
                for j in range(EPC):
                    for i in range(NCHJ[j]):
                        off = i * P
                        csz = min(P, CJ[j] - off)
                        wbps = ps_pool.tile([P, P], f32, name=f"wbps{j}_{off}", tag="A1", bufs=4)
                        nc.tensor.transpose(
                            out=wbps[:, :csz],
                            in_=wsall[:csz, cc:cc + 1].to_broadcast([csz, P]),
                            identity=ident[0:csz, 0:csz],
                        )
                        nc.vector.tensor_copy(wb[j][:, off:off + csz], wbps[:, :csz])
                        cc += 1

            def emit_upgate(j, interleave=None):
                xg_t = [xgb[j][:, k * CJ[j]:(k + 1) * CJ[j]] for k in range(KH)]
                g_t = [act_pool.tile([P, CJ[j]], f32, name=f"g{j}_{m}", tag="gtmp", bufs=3) for m in range(MI)]
                for m in range(MI):
                    if interleave is not None:
                        interleave(m)
                    wgub = wst_pool.tile([P, 2 * KH * P], bf16, name=f"wgub{j}_{m}", tag="wblk", bufs=4)
                    nc.sync.dma_start(wgub[:], wgu_h[j][m * P:(m + 1) * P, :])
                    wgb = [wgub[:, k * P:(k + 1) * P] for k in range(KH)]
                    wub = [wgub[:, (KH + k) * P:(KH + k + 1) * P] for k in range(KH)]
                    for (coff, csz) in CHJ[j]:
                        psg = ps_pool.tile([P, csz], f32, name=f"psg{j}_{m}_{coff}", tag="A1", bufs=4)
                        for k in range(KH):
                            nc.tensor.matmul(psg[:], lhsT=wgb[k],
                                             rhs=xg_t[k][:, coff:coff + csz],
                                             start=(k == 0), stop=(k == KH - 1))
                        nc.scalar.activation(g_t[m][:, coff:coff + csz], psg[:], SILU, bias=zbias[:])
                        psu = ps_pool.tile([P, csz], f32, name=f"psu{j}_{m}_{coff}", tag="A1", bufs=4)
                        for k in range(KH):
                            nc.tensor.matmul(psu[:], lhsT=wub[k],
                                             rhs=xg_t[k][:, coff:coff + csz],
                                             start=(k == 0), stop=(k == KH - 1))
                        # a = silu(g) * u straight out of PSUM, rounded to bf16
                        nc.vector.tensor_mul(out=a_t[j][m][:, coff:coff + csz],
                                             in0=g_t[m][:, coff:coff + csz], in1=psu[:])

            def emit_down(j, interleave=None):
                for mg in range(MH // 2):
                    if interleave is not None:
                        interleave(mg)
                    wdb = dst_pool.tile([P, 2 * KI * P], bf16, name=f"wdb{j}_{mg}", tag="wdb", bufs=4)
                    nc.sync.dma_start(wdb[:], wd_h[j][mg * P:(mg + 1) * P, :])
                    # the final groups' writes go via sync's hardware DGE so
                    # the end-of-kernel gpsimd drain isn't waiting on SWDGE
                    weng = nc.sync if (j == 1 and mg >= MH // 2 - 2) else nc.gpsimd
                    for mh in range(2):
                        m = mg * 2 + mh
                        for (coff, csz) in CHJ[j]:
                            psz = ps_pool.tile([P, csz], f32, name=f"psz{j}_{m}_{coff}", tag="A1", bufs=4)
                            for k in range(KI):
                                nc.tensor.matmul(psz[:], lhsT=wdb[:, (mh * KI + k) * P:(mh * KI + k + 1) * P],
                                                 rhs=a_t[j][k][:, coff:coff + csz],
                                                 start=(k == 0), stop=(k == KI - 1))
                            zst = stage_pool.tile([P, csz], bf16, name=f"zst{j}_{m}_{coff}", tag="zst", bufs=2)
                            # combine-weight scaling fused into the eviction
                            nc.vector.tensor_mul(out=zst[:], in0=wb[j][:, coff:coff + csz], in1=psz[:])
                            weng.dma_start(
                                zt_h[m * P:(m + 1) * P, COFFJ[j] + coff:COFFJ[j] + coff + csz], zst[:])

            def load_sgub(mi):
                # gate and up halves as separate tiles/DMAs so the gate-proj
                # matmuls only wait on the first 0.5MB
                sgub_g = sst_pool.tile([P, KH * P], bf16, name=f"sgubg{mi}", tag="ssbg", bufs=3)
                nc.sync.dma_start(sgub_g[:], swgu_h[mi * P:(mi + 1) * P, 0:KH * P])
                sgub_u = sst_pool.tile([P, KH * P], bf16, name=f"sgubu{mi}", tag="ssbu", bufs=3)
                nc.sync.dma_start(sgub_u[:], swgu_h[mi * P:(mi + 1) * P, KH * P:2 * KH * P])
                return sgub_g, sgub_u

            def emit_shared_ug(mi, pre=None):
                sgub_g, sgub_u = pre if pre is not None else load_sgub(mi)
                psgs = ps_pool.tile([P, T], f32, name=f"psgs{mi}", tag="B1", bufs=2)
                for k in range(KH):
                    for n in range(NT):
                        nc.tensor.matmul(psgs[:, n * 512:(n + 1) * 512],
                                         lhsT=sgub_g[:, k * P:(k + 1) * P],
                                         rhs=xt_t[k][:, n * 512:(n + 1) * 512],
                                         start=(k == 0), stop=(k == KH - 1))
                nc.scalar.activation(sg_t[mi][:], psgs[:], SILU, bias=zbias[:])
                psus = ps_pool.tile([P, T], f32, name=f"psus{mi}", tag="B1", bufs=2)
                for k in range(KH):
                    for n in range(NT):
                        nc.tensor.matmul(psus[:, n * 512:(n + 1) * 512],
                                         lhsT=sgub_u[:, k * P:(k + 1) * P],
                                         rhs=xt_t[k][:, n * 512:(n + 1) * 512],
                                         start=(k == 0), stop=(k == KH - 1))
                nc.vector.tensor_mul(out=as_t[mi][:], in0=sg_t[mi][:], in1=psus[:])

            def emit_shared_down(ms):
                for mg in ms:
                    sdb = sst_pool.tile([P, 4 * KS * P], bf16, name=f"sdb{mg}", tag="sdb", bufs=2)
                    nc.sync.dma_start(sdb[:], swd_h[mg * P:(mg + 1) * P, :])
                    for mh in range(4):
                        m = mg * 4 + mh
                        psys = ps_pool.tile([P, T], f32, name=f"psys{m}", tag="B1", bufs=2)
                        for ki in range(KS):
                            for n in range(NT):
                                nc.tensor.matmul(psys[:, n * 512:(n + 1) * 512],
                                                 lhsT=sdb[:, (mh * KS + ki) * P:(mh * KS + ki + 1) * P],
                                                 rhs=as_t[ki][:, n * 512:(n + 1) * 512],
                                                 start=(ki == 0), stop=(ki == KS - 1))
                        sstg = stage_pool.tile([P, T], bf16, name=f"sstg{m}", tag="sstage", bufs=2)
                        # alternate the PSUM eviction engine so neither
                        # scalar nor vector saturates during the tail
                        if m % 2 == 0:
                            nc.scalar.copy(sstg[:], psys[:])
                        else:
                            nc.vector.tensor_copy(sstg[:], psys[:])
                        nc.sync.dma_start(st_h[m * P:(m + 1) * P, :], sstg[:])

            # PE-section order: start with shared_ug(0), whose k-loop
            # pipelines against the granular xt stream (PE starts after just
            # ~1MB: the sgub gate-half + first xt granule).  gate follows (xt
            # fully resident by then) so its softmax -> DRAM -> indirect-
            # gather chain drains on gpsimd/vector during upgate(0), making
            # the gather's PE transposes free.  PE-light shared sections are
            # interleaved into the DMA-heavy routed sections to keep the
            # weight-stream demand under the HBM rate.
            pre0 = load_sgub(0)
            for kk in range(KH // 2):
                load_xt(kk)
                if kk == 3:
                    nc.sync.dma_start(xgb[0][:], xg_h[0][:])
            nc.sync.dma_start(xgb[1][:], xg_h[1][:])
            emit_shared_ug(0, pre=pre0)
            emit_gate()
            emit_upgate(0)
            emit_gather()
            emit_down(0, interleave=lambda mg: emit_shared_ug(1) if mg == 2 else None)
            emit_upgate(1, interleave=lambda m: emit_shared_ug(2) if m == 5 else None)
            emit_down(1, interleave=lambda mg: emit_shared_down([mg]) if mg < 4 else None)

    nc.compile()
    return nc


def _get_nc(CA, CB):
    if (CA, CB) not in _NC_CACHE:
        _NC_CACHE[(CA, CB)] = _build(CA, CB)
    return _NC_CACHE[(CA, CB)]


def kernel(**inputs):
    global LAST_RESULTS
    from concourse.bass_utils import run_bass_kernel_spmd

    BF16 = _bf16()

    hs = np.asarray(inputs["hidden_states"], dtype=np.float32)
    gate_w = np.asarray(inputs["gate_w"], dtype=np.float32)
    w_gate = np.asarray(inputs["w_gate"], dtype=np.float32)
    w_up = np.asarray(inputs["w_up"], dtype=np.float32)
    w_down = np.asarray(inputs["w_down"], dtype=np.float32)
    sw_gate = np.asarray(inputs["sw_gate"], dtype=np.float32)
    sw_up = np.asarray(inputs["sw_up"], dtype=np.float32)
    sw_down = np.asarray(inputs["sw_down"], dtype=np.float32)

    orig_shape = hs.shape
    x = hs.reshape(-1, H)
    assert x.shape[0] == T

    # ---- host: discrete routing only (top-4 selection + dispatch tables) ----
    logits = x @ gate_w.T
    smax = logits.max(axis=-1, keepdims=True)
    sc = np.exp(logits - smax)
    sc /= sc.sum(axis=-1, keepdims=True)
    order = np.argsort(-sc, axis=-1, kind="stable")[:, :TOPK]
    mask = np.zeros((T, E), dtype=np.float32)
    mask[np.arange(T)[:, None], order] = 1.0
    tok_lists = [np.nonzero(mask[:, e])[0].astype(np.int64) for e in range(E)]

    # balance: pair the i-th most-loaded expert with the i-th least-loaded
    sizes = np.array([len(tk) for tk in tok_lists])
    by_load = np.argsort(-sizes, kind="stable")
    pairs = [(int(by_load[i]), int(by_load[E - 1 - i])) for i in range(NCORES)]
    CA = max(64, int(np.ceil(max(sizes[p[0]] for p in pairs) / 32)) * 32)
    CB = max(64, int(np.ceil(max(sizes[p[1]] for p in pairs) / 32)) * 32)
    CJ = [CA, CB]

    nc = _get_nc(CA, CB)

    xT = np.ascontiguousarray(x.T)
    xTb = xT.astype(BF16)
    # xt packed for 2-ktile row loads: xtp[kk*P + p, a*T + t] = x[t, (2kk+a)*P + p]
    xtp = _group_rows(xTb, KH, T, 2)
    # gate weights packed: gwtb[p, k*E + e] = gate_w[e, k*P + p]
    gwtb = np.ascontiguousarray(
        gate_w.T.reshape(KH, P, E).transpose(1, 0, 2).reshape(P, KH * E)).astype(BF16)
    # mask packed: maskb[p, t8*E + e] = mask[t8*P + p, e]
    maskb = np.ascontiguousarray(mask.reshape(T // P, P, E).transpose(1, 0, 2).reshape(P, (T // P) * E))

    # shared slices, zero-padded to 384 and tile-major packed
    def pad_cols(w, newc):
        out = np.zeros((w.shape[0], newc), dtype=np.float32)
        out[:, :w.shape[1]] = w
        return out

    def pad_rows(w, newr):
        out = np.zeros((newr, w.shape[1]), dtype=np.float32)
        out[:w.shape[0], :] = w
        return out

    nchA = (CA + P - 1) // P
    nchB = (CB + P - 1) // P
    in_maps = []
    for c in range(NCORES):
        es = pairs[c]
        # widx[p, cc] = flat wflat row for slot cc*P + p of the concatenated
        # (expert A chunks, expert B chunks) compact batch
        widx = np.full((P, nchA + nchB), ZERO_ROW_FLAT, dtype=np.int32)
        sg_p = _pack_st(pad_cols(sw_gate[:, c * ISS:(c + 1) * ISS], ISSP), KH, KS)
        su_p = _pack_st(pad_cols(sw_up[:, c * ISS:(c + 1) * ISS], ISSP), KH, KS)
        im = {
            "xt": xtp, "gwtb": gwtb, "maskb": maskb, "widx": widx,
            "swgu": _concat_cols([sg_p, su_p], KS, KH * P).astype(BF16),
            "swd": _group_rows(
                _pack_st(pad_rows(sw_down[c * ISS:(c + 1) * ISS, :], ISSP), KS, MH),
                MH, KS * P, 4).astype(BF16),
        }
        for j, e in enumerate(es):
            tk = tok_lists[e]
            C = CJ[j]
            ccoff = 0 if j == 0 else nchA
            flat = (tk * E + e).astype(np.int32)
            for s in range(len(tk)):
                widx[s % P, ccoff + s // P] = flat[s]
            # gathered activations, tile-major: xg[p, k*C + c] = x[tok_c, k*P + p]
            xg = np.zeros((P, KH * C), dtype=BF16)
            g = xTb[:, tk].reshape(KH, P, len(tk)).transpose(1, 0, 2)  # [P, KH, n]
            xg.reshape(P, KH, C)[:, :, :len(tk)] = g
            im[f"xg{j}"] = xg
            wg_p = _pack_st(w_gate[e], KH, MI)
            wu_p = _pack_st(w_up[e], KH, MI)
            im[f"wgu{j}"] = _concat_cols([wg_p, wu_p], MI, KH * P).astype(BF16)
            im[f"wd{j}"] = _group_rows(_pack_st(w_down[e], KI, MH), MH, KI * P, 2).astype(BF16)
        in_maps.append(im)

    trace = bool(int(os.environ.get("BASSMOE_TRACE", "0")))
    kwargs = {}
    if trace:
        kwargs = dict(trace=True, tmpdir=os.environ.get("BASSMOE_TRACE_DIR") or None)
        tcores = os.environ.get("BASSMOE_TRACE_CORES")
        if tcores:
            kwargs["trace_cores"] = [int(x) for x in tcores.split(",")]
            kwargs["stitch_traces"] = False
    res = run_bass_kernel_spmd(nc, in_maps, core_ids=list(range(NCORES)), **kwargs)
    LAST_RESULTS = res

    # ---- host: unshard (scatter-add compact expert outputs + sum partials) ----
    y = np.zeros((T, H), dtype=np.float64)
    st_sum = np.zeros((H, T), dtype=np.float64)
    for c in range(NCORES):
        r = res.results[c]
        st_sum += np.asarray(r["st"], dtype=np.float64)
        coff = 0
        for j, e in enumerate(pairs[c]):
            tk = tok_lists[e]
            zt = np.asarray(r["zt"], dtype=np.float64)
            y[tk] += zt[:, coff:coff + len(tk)].T
            coff += CJ[j]
    y += st_sum.T
    return y.astype(np.float32).reshape(orig_shape)
